# revision 37
# baseline (speedup 1.0000x reference)
"""Multi-head attention (GQA prefill with KV cache) on 8 trn2 NeuronCores.

Sharding: tensor-parallel over heads. Core m owns KV head m (of 8) and the
4 query heads 4m..4m+3.  Each core computes its heads' attention output and
a partial x @ wo.T contribution; the host sums the 8 partials.

Layout notes:
  - All activations on device are "feature-major" ([feature, token]) so the
    token dim rides the matmul moving dim; x is transposed on the host once.
  - RoPE's (even, odd) pair interleave is removed by permuting wq/wk rows and
    cache_k's head_dim on the host (QK^T is invariant to a shared permutation
    of head_dim), so on device RoPE is plain 64-partition block arithmetic.
  - Scores are computed transposed ([key, query]) so the softmax denominator
    is an all-ones matmul and the attention output lands feature-major,
    which feeds the wo matmul directly.

Pipeline notes (v2):
  - PSUM: four independent 1-bank tags (qk / sc / out / wo), bufs=2 each,
    so the three phases never serialize on pool rotation.
  - Softmax denominators accumulate on the (otherwise idle) DVE in fp16;
    a single all-ones [128,128] matmul broadcasts the per-query sums to all
    partitions, replacing 16 ones-vector matmuls per attention unit.
  - Emission is software-pipelined at sub-unit granularity: projection and
    wo-projection matmul groups are interleaved between attention kt-steps,
    keeping PE busy while the exp stream (ACT-paced) advances.
  - Output partials are written bf16, one DMA per 128-token row.
"""

import os
import sys

import numpy as np

if "/opt/trn_rl_repo" not in sys.path:
    sys.path.insert(0, "/opt/trn_rl_repo")

import ml_dtypes

import concourse.bass as bass
import concourse.mybir as mybir
import concourse.tile as tile
from concourse.bass_utils import run_bass_kernel_spmd
from concourse.masks import make_identity

BF16 = mybir.dt.bfloat16
FP16 = mybir.dt.float16
F32 = mybir.dt.float32
NP_BF16 = ml_dtypes.bfloat16

B, S, DIM = 4, 1024, 4096
N_HEADS, N_KV_HEADS = 32, 8
HD = 128
PAST = 1024
NCORES = 8
NQ = N_HEADS // NCORES  # 4 q heads per core
T = B * S  # 4096 tokens
DT = DIM // 128  # 32 contraction tiles
CH = 512  # projection token-chunk
NCH_B = S // CH  # chunks per batch (2)
SCW = 512  # attention s-chunk width
NKT = (PAST + S) // 128  # 16 key tiles per batch
ISQRT_HD = 1.0 / float(np.sqrt(HD))

LAST_EXEC_NS = None
LAST_RESULTS = None

_CACHED = {}


def _split_multi_waits(nc):
    """walrus' per-instruction sync encoding fits one wait; hoist extras
    onto standalone EventSemaphore instructions on the same engine queue."""
    for f in nc.m.functions:
        for blk in f.blocks:
            insts = blk.instructions
            if not any(i.sync_info and i.sync_info.on_wait
                       and len(i.sync_info.on_wait) > 1 for i in insts):
                continue
            new = []
            for inst in insts:
                si = inst.sync_info
                if si is not None and si.on_wait and len(si.on_wait) > 1:
                    waits = list(si.on_wait)
                    for wt in waits[:-1]:
                        evs = mybir.InstEventSemaphore(
                            name=f"I-wsplit-{nc.next_id()}", ins=[], outs=[])
                        evs.engine = inst.engine
                        evs.sync_info = mybir.SyncInfo(on_wait=[wt],
                                                       on_update=[])
                        new.append(evs)
                    inst.sync_info = mybir.SyncInfo(
                        on_wait=[waits[-1]],
                        on_update=list(si.on_update or []))
                new.append(inst)
            insts[:] = new


def _build_nc(split_waits=True, mode="full", reps=1):
    nc = bass.Bass("TRN2", target_bir_lowering=False, debug=False,
                   num_devices=NCORES)

    xt = nc.dram_tensor("xt", [DIM, T], BF16, kind="ExternalInput")
    wqt = nc.dram_tensor("wqt", [DIM, NQ * HD], BF16, kind="ExternalInput")
    wkt = nc.dram_tensor("wkt", [DIM, HD], BF16, kind="ExternalInput")
    wvt = nc.dram_tensor("wvt", [DIM, HD], BF16, kind="ExternalInput")
    wot = nc.dram_tensor("wot", [NQ * HD, DIM], BF16, kind="ExternalInput")
    ckt = nc.dram_tensor("ckt", [B, HD, PAST], BF16, kind="ExternalInput")
    cv = nc.dram_tensor("cv", [B, PAST, HD], BF16, kind="ExternalInput")
    cos = nc.dram_tensor("cos", [HD // 2, S], BF16, kind="ExternalInput")
    sin = nc.dram_tensor("sin", [HD // 2, S], BF16, kind="ExternalInput")
    out_p = nc.dram_tensor("out_p", [T, DIM], BF16, kind="ExternalOutput")

    with tile.TileContext(nc) as tc:
        if mode == "null":
            _emit_null(tc, nc)
        else:
            for _ in range(reps):
                _emit(tc, nc, xt, wqt, wkt, wvt, wot, ckt, cv, cos, sin,
                      out_p, mode=mode)
    if split_waits:
        _split_multi_waits(nc)
    return nc


def _emit_null(tc, nc):
    """Near-empty program: measures per-dispatch overhead in bench.py."""
    from contextlib import ExitStack
    with ExitStack() as ctx:
        cw = ctx.enter_context(tc.tile_pool(name="nullp", bufs=1))
        t = cw.tile([128, 128], BF16, name="null_t")
        nc.vector.memset(t, 0.0)


def _emit(tc, nc, xt, wqt, wkt, wvt, wot, ckt, cv, cos, sin, out_p,
          mode="full"):
    from contextlib import ExitStack
    do_attn = mode in ("full", "bc")
    do_wo = mode in ("full", "bd")

    with ExitStack() as ctx:
        cw = ctx.enter_context(tc.tile_pool(name="consts", bufs=1))
        pb = ctx.enter_context(tc.tile_pool(name="perbatch", bufs=2))
        wk = ctx.enter_context(tc.tile_pool(name="work", bufs=2))
        ps = ctx.enter_context(tc.tile_pool(name="ps", bufs=2, space="PSUM"))

        # ---- resident constants -------------------------------------------
        # DMA order is startup-critical: the first emitted work is the k/v
        # projection of batch 0 chunk 0, which needs only wkt/wvt (+ the xt
        # chunk, issued inside proj_batch).  wqt follows (needed ~15us later
        # by the first q unit); wot and the kv-cache much later.
        # Weights/trig/b0-cache ride the SECOND hwdge queue (Activation —
        # idle at startup) so they transfer in parallel with the xt stream
        # on the SP queue, halving the startup DMA serialization.
        wkt_sb = cw.tile([128, DT * HD], BF16, name="wkt_sb")
        nc.scalar.dma_start(
            out=wkt_sb.rearrange("p (n j) -> p n j", n=DT),
            in_=wkt[:, :].rearrange("(n p) j -> p n j", p=128))
        wvt_sb = cw.tile([128, DT * HD], BF16, name="wvt_sb")

        def load_wvt():
            nc.scalar.dma_start(
                out=wvt_sb.rearrange("p (n j) -> p n j", n=DT),
                in_=wvt[:, :].rearrange("(n p) j -> p n j", p=128))
        # cos/sin duplicated across both 64-partition halves so RoPE's two
        # multiplies can run full-width: q*cos gives (r*cos | i*cos).
        # (loaded after the first xt chunk — see proj_batch)
        cos_sb = cw.tile([128, S], BF16, name="cos_sb")
        sin_sb = cw.tile([128, S], BF16, name="sin_sb")

        def load_cos_sin():
            nc.scalar.dma_start(out=cos_sb[0:64, :], in_=cos[:, :])
            nc.scalar.dma_start(out=cos_sb[64:128, :], in_=cos[:, :])
            nc.scalar.dma_start(out=sin_sb[0:64, :], in_=sin[:, :])
            nc.scalar.dma_start(out=sin_sb[64:128, :], in_=sin[:, :])

        ones128 = cw.tile([128, 128], FP16, name="ones128")
        nc.vector.memset(ones128, 1.0)
        ident = cw.tile([128, 128], BF16, name="ident")
        make_identity(nc, ident)
        wqt_sb = cw.tile([128, DT * NQ * HD], BF16, name="wqt_sb")
        wot_sb = cw.tile([128, NQ * DIM], BF16, name="wot_sb")

        def load_wqt():
            nc.scalar.dma_start(
                out=wqt_sb.rearrange("p (n j) -> p n j", n=DT),
                in_=wqt[:, :].rearrange("(n p) j -> p n j", p=128))

        def load_wot():
            nc.scalar.dma_start(
                out=wot_sb.rearrange("p (n d) -> p n d", n=NQ),
                in_=wot[:, :].rearrange("(n p) d -> p n d", p=128))

        # ---- per-batch / work tiles (slot handles, rotated via tags) ------
        def batch_tiles(b):
            qb_t = pb.tile([128, NQ * S], BF16, name="qb", tag="qb")
            kb_t = pb.tile([128, S], BF16, name="kb", tag="kb")
            vb_t = pb.tile([128, S], BF16, name="vb", tag="vb")
            attnb_t = pb.tile([128, NQ * S], BF16, name="attnb", tag="attnb",
                              bufs=1)
            ckt_b = pb.tile([128, PAST], BF16, name="ckt_b", tag="ckt_b")
            cv_b = pb.tile([128, PAST], BF16, name="cv_b", tag="cv_b")
            return dict(qb=qb_t, kb=kb_t, vb=vb_t, attnb=attnb_t,
                        ckt=ckt_b, cv=cv_b)

        def rope(dst_tile, dst_col, src_ps, cosc, sinc, n):
            """src layout (r|i) on partition halves.
            dst[0:64] = r*cos - i*sin ; dst[64:128] = r*sin + i*cos.
            A fast ACT copy stages src to fp16 SBUF first, freeing the "qk"
            PSUM slot immediately (the DVE rope ops otherwise queue behind
            attention acc-adds and stall the next proj group's matmuls).
            Two full-width muls: tc = (r*cos | i*cos), ts = (r*sin | i*sin),
            then dst_r = tc_hi - ts_lo, dst_i = ts_hi + tc_lo."""
            qraw = wk.tile([128, CH], FP16, name="qraw", tag="qraw", bufs=2)
            nc.scalar.copy(qraw[:, :n], src_ps)
            tc_ = wk.tile([128, CH], FP16, name="rope_tc", tag="rope_tc",
                          bufs=1)
            ts_ = wk.tile([128, CH], FP16, name="rope_ts", tag="rope_ts",
                          bufs=1)
            nc.vector.tensor_mul(tc_[:, :n], qraw[:, :n], cosc)
            nc.vector.tensor_mul(ts_[0:64, :n], qraw[64:128, :n],
                                 sinc[64:128, :])
            nc.vector.tensor_mul(ts_[64:128, :n], qraw[0:64, :n],
                                 sinc[0:64, :])
            nc.vector.tensor_sub(dst_tile[0:64, dst_col:dst_col + n],
                                 tc_[0:64, :n], ts_[0:64, :n])
            nc.vector.tensor_add(dst_tile[64:128, dst_col:dst_col + n],
                                 ts_[64:128, :n], tc_[64:128, :n])

        # ================= work-unit generators (each yields steps) ========
        def xt_load(b, c):
            """DMA one 512-token chunk of x into SBUF (feature-major).
            Split into two half-d DMAs so the projection's first 16 accum
            matmuls can start as soon as the first half lands (subtile deps)."""
            xt_t = wk.tile([128, DT * CH], BF16, name="xt_t", tag="xt")
            p0 = c * CH
            # First chunk is latency-critical (PE idles until d0.. land):
            # progressive split so the first accum matmuls start ~7us in.
            splits = [2, 2, 4, 8, 16] if (b == 0 and c == 0) else [16, 16]
            d0 = 0
            for step in splits:
                nc.sync.dma_start(
                    out=xt_t[:, d0 * CH:(d0 + step) * CH].rearrange(
                        "p (n t) -> p n t", n=step),
                    in_=xt[128 * d0:128 * (d0 + step),
                           b * S + p0: b * S + p0 + CH].rearrange(
                        "(n p) t -> p n t", p=128))
                d0 += step
            return xt_t

        def proj_unit(bt, b, c, kind, xt_t, alt=False):
            """One projection group: 32 accum matmuls + rope (or v-transpose).
            kind: 0..3 = q head j, 4 = k, 5 = v. Yields 5 steps.
            alt: prologue groups run back-to-back with no filler, so they
            alternate onto the (then idle) "out" bank instead of
            serializing on the single "qk" bank."""
            p0 = c * CH
            cosc = cos_sb[:, p0:p0 + CH]
            sinc = sin_sb[:, p0:p0 + CH]
            tag = "out" if (alt and kind % 2 == 1) else "qk"
            acc_ps = ps.tile([128, CH], F32, name="proj_ps", tag=tag,
                             bufs=2 if tag == "out" else 1)
            if kind < NQ:
                w_sb, wof = wqt_sb, kind * HD
                wstride = NQ * HD
            elif kind == NQ:
                w_sb, wof, wstride = wkt_sb, 0, HD
            else:
                w_sb, wof, wstride = wvt_sb, 0, HD
            for d0 in range(0, DT, 8):
                for d in range(d0, d0 + 8):
                    nc.tensor.matmul(
                        acc_ps,
                        lhsT=w_sb[:, d * wstride + wof:
                                  d * wstride + wof + HD],
                        rhs=xt_t[:, d * CH:(d + 1) * CH],
                        start=(d == 0), stop=(d == DT - 1))
                yield  # ~8 x 213ns PE quantum
            if kind < NQ:
                rope(bt["qb"], kind * S + p0, acc_ps, cosc, sinc, CH)
            elif kind == NQ:
                rope(bt["kb"], p0, acc_ps, cosc, sinc, CH)
            else:
                vcp = wk.tile([128, CH], BF16, name="vcp", tag="vcp", bufs=1)
                nc.scalar.copy(vcp, acc_ps)
                for tsub in range(CH // 128):
                    vtr_ps = ps.tile([128, 128], BF16, name="vtr_ps",
                                     tag="sc")
                    nc.tensor.transpose(vtr_ps,
                                        vcp[:, tsub * 128:(tsub + 1) * 128],
                                        ident)
                    col = (c * (CH // 128) + tsub) * 128
                    nc.vector.tensor_copy(bt["vb"][:, col:col + 128], vtr_ps)
            yield

        def attn_unit(bt, b, sc, h, epi_out):
            """Attention for (query-chunk sc, head h): 16 kt steps.
            Scores land [key, query] in PSUM; exp on ACT; denominator
            accumulates on DVE in fp16; all-ones matmul broadcasts sums.

            Pipelining (v3, NTFF-driven):
              - av(kt) is emitted one step behind scores(kt+1), so the PE
                never waits on the ACT exp stream mid-unit.
              - The epilogue (sums broadcast + reciprocal + normalize) is
                handed back via epi_out and emitted ~2 kt-steps into the
                NEXT unit, so its ACT copy does not delay exp(kt0') and the
                sums matmuls get runway before reading the DVE accs."""
            s0 = sc * SCW
            out_ps = ps.tile([128, SCW], F32, name="out_ps", tag="out")
            acc0 = wk.tile([128, SCW], FP16, name="acc0", tag="acc0", bufs=1)
            acc1 = wk.tile([128, SCW], FP16, name="acc1", tag="acc1", bufs=1)
            qsl = bt["qb"][:, h * S + s0:h * S + s0 + SCW]

            def kv(kt):
                if kt < PAST // 128:
                    return (bt["ckt"][:, kt * 128:(kt + 1) * 128],
                            bt["cv"][:, kt * 128:(kt + 1) * 128])
                kn = kt - PAST // 128
                return (bt["kb"][:, kn * 128:(kn + 1) * 128],
                        bt["vb"][:, kn * 128:(kn + 1) * 128])

            # kt-steps processed in PAIRS sharing one 2-bank PSUM tile so a
            # single [128,1024] exp covers both — halves ACT instruction
            # count/overhead (the exp stream otherwise paces attention).
            prev = None
            for p in range(NKT // 2):
                k0, v0 = kv(2 * p)
                k1, v1 = kv(2 * p + 1)
                sc_ps = ps.tile([128, 2 * SCW], F32, name="sc_ps", tag="sc")
                nc.tensor.matmul(sc_ps[:, 0:SCW], lhsT=k0, rhs=qsl)
                nc.tensor.matmul(sc_ps[:, SCW:2 * SCW], lhsT=k1, rhs=qsl)
                if prev is not None:
                    pexp, pv0, pv1 = prev
                    nc.tensor.matmul(out_ps, lhsT=pv0, rhs=pexp[:, 0:SCW],
                                     start=(p == 1), stop=False)
                    nc.tensor.matmul(out_ps, lhsT=pv1,
                                     rhs=pexp[:, SCW:2 * SCW],
                                     start=False, stop=False)
                exp_t = wk.tile([128, 2 * SCW], BF16, name="exp_t",
                                tag="exp", bufs=2)
                nc.scalar.activation(exp_t, sc_ps,
                                     mybir.ActivationFunctionType.Exp,
                                     scale=ISQRT_HD)
                if p == 0:
                    nc.vector.tensor_copy(acc0, exp_t[:, 0:SCW])
                    nc.vector.tensor_copy(acc1, exp_t[:, SCW:2 * SCW])
                else:
                    nc.vector.tensor_add(acc0, acc0, exp_t[:, 0:SCW])
                    nc.vector.tensor_add(acc1, acc1, exp_t[:, SCW:2 * SCW])
                prev = (exp_t, v0, v1)
                yield
            pexp, pv0, pv1 = prev
            nc.tensor.matmul(out_ps, lhsT=pv0, rhs=pexp[:, 0:SCW],
                             start=False, stop=False)
            nc.tensor.matmul(out_ps, lhsT=pv1, rhs=pexp[:, SCW:2 * SCW],
                             start=False, stop=True)

            def epilogue():
                sums_ps = ps.tile([128, SCW], F32, name="sums_ps", tag="sc")
                nc.tensor.matmul(sums_ps, lhsT=ones128, rhs=acc0,
                                 start=True, stop=False)
                nc.tensor.matmul(sums_ps, lhsT=ones128, rhs=acc1,
                                 start=False, stop=True)
                # Fast ACT copy frees the "sc" PSUM slot quickly; the
                # reciprocal (3.4us on real DVE) runs in-place afterwards,
                # off the PE-critical path — quartered so the first attnb
                # 128-token tile lands ~3us sooner (wo rows consume attnb
                # per 128-token tile and otherwise stall on batch 3's tail).
                inv_bc = wk.tile([128, SCW], BF16, name="inv_bc", tag="inv",
                                 bufs=1)
                nc.scalar.copy(inv_bc, sums_ps)
                for qtr in range(SCW // 128):
                    sl = slice(qtr * 128, (qtr + 1) * 128)
                    with nc.allow_low_precision(reason="softmax denom recip"):
                        nc.vector.reciprocal(inv_bc[:, sl], inv_bc[:, sl])
                    nc.vector.tensor_mul(
                        bt["attnb"][:, h * S + s0 + qtr * 128:
                                    h * S + s0 + (qtr + 1) * 128],
                        out_ps[:, sl], inv_bc[:, sl])
            epi_out.append(epilogue)
            yield

        NDC = DIM // SCW  # 8 wo column-groups per token tile
        HROW = NDC // 4   # groups per staging tile (quarter row)

        def wo_unit(bt, b, tt, dc, st_half, alt):
            """Partial x@wo.T for token-tile tt, 512 output dims dc.
            alt: when the proj fillers can no longer touch "qk" (all of the
            last batch, each batch's final chunk), odd groups borrow that
            idle bank so back-to-back wo groups don't serialize on the
            single "wo" bank waiting for the staging copy."""
            tag = "qk" if (alt and dc % 2 == 1) else "wo"
            wo_ps = ps.tile([128, SCW], F32, name="wo_ps", tag=tag,
                            bufs=1)
            for j in range(NQ):
                nc.tensor.matmul(
                    wo_ps,
                    lhsT=bt["attnb"][:, j * S + tt * 128:
                                     j * S + (tt + 1) * 128],
                    rhs=wot_sb[:, j * DIM + dc * SCW:
                               j * DIM + (dc + 1) * SCW],
                    start=(j == 0), stop=(j == NQ - 1))
            # GPSIMD cannot read PSUM.  All staging copies go to ACT: with
            # paired exps ACT is ~45% loaded, while the DVE queue carries
            # the epilogue reciprocal bursts that would delay the copy and
            # stall the single-banked wo_ps slot (measured via NTFF).
            dst = st_half[:, (dc % HROW) * SCW:(dc % HROW + 1) * SCW]
            nc.scalar.copy(dst, wo_ps)
            if dc % HROW == HROW - 1:
                row = (b * (S // 128) + tt) * 128
                col = (dc // HROW) * HROW * SCW
                nc.sync.dma_start(
                    out=out_p[row:row + 128, col:col + HROW * SCW],
                    in_=st_half)
            yield

        def wo_row(bt, b, tt, alt):
            for part in range(NDC // HROW):
                st_half = wk.tile([128, HROW * SCW], BF16, name="st_half",
                                  tag="st", bufs=2)
                for dc in range(part * HROW, (part + 1) * HROW):
                    yield from wo_unit(bt, b, tt, dc, st_half, alt)

        # ================= scheduler ======================================
        # Per batch: proj(b) is fully emitted before attn(b).  attn(b)'s 8
        # units are ACT-paced; between kt-steps we interleave filler steps
        # from wo(b) rows (ready once all heads of their s-chunk are done)
        # and proj(b+1) (+ its xt prefetch).  All wo(b) rows are emitted
        # before attn(b+1) starts so attnb can be single-buffered.
        bts = [None] * B
        xts = [None] * B

        def cache_load(b):
            # b0's cache rides the ACT queue (idle during the prologue);
            # later batches stay on SP so the issue cost doesn't delay the
            # busy exp/staging stream on ACT.
            eng = nc.scalar if b == 0 else nc.sync
            bt = bts[b]
            eng.dma_start(out=bt["ckt"], in_=ckt[b, :, :])
            eng.dma_start(
                out=bt["cv"].rearrange("p (n d) -> p n d", n=PAST // 128),
                in_=cv[b, :, :].rearrange("(n p) t -> p n t", p=128))

        def proj_batch(b):
            """Generator over proj(b) steps (incl. xt loads + cache DMA).
            The kv-cache DMA is issued after the first xt chunk so it does
            not delay the projection-critical loads on the in-order queue.
            Heads q2/q3 are deferred (see projq23_batch): they are only
            needed by attention units h2/h3, so they serve as attn-phase
            filler — which matters for the last batch, whose attention has
            no next-batch projections to interleave."""
            bts[b] = batch_tiles(b)
            xts[b] = [None] * NCH_B
            for c in range(NCH_B):
                xts[b][c] = xt_load(b, c)
                if b == 0:
                    if c == 0:
                        load_wvt()
                        load_cos_sin()
                        load_wqt()
                    else:
                        load_wot()
                        cache_load(b)
                elif c == 0:
                    cache_load(b)
                for kind in (NQ, NQ + 1, 0, 1):  # k, v first
                    yield from proj_unit(bts[b], b, c, kind, xts[b][c],
                                         alt=(b == 0))

        def projq23_batch(b):
            for c in range(NCH_B):
                for kind in (2, 3):
                    yield from proj_unit(bts[b], b, c, kind, xts[b][c])

        def attn_batch(b):
            epis = []        # deferred epilogue of the previous unit
            pending_sc = None  # sc-chunk whose last epilogue is deferred
            for sc in range(S // SCW):
                for h in range(NQ if do_attn else 0):
                    step_n = 0
                    for _ in attn_unit(bts[b], b, sc, h, epis):
                        yield
                        step_n += 1
                        if step_n == 2 and epis:
                            epis.pop(0)()
                            if pending_sc is not None:
                                yield ("sc_done", pending_sc)
                                pending_sc = None
                pending_sc = sc
            while epis:
                epis.pop(0)()
            if pending_sc is not None:
                yield ("sc_done", pending_sc)

        def wo_rows(b, tts, alt=False):
            for tt in tts:
                yield from wo_row(bts[b], b, tt, alt)

        # prologue: batch 0 projections (k/v/q0/q1), no filler available
        for _ in proj_batch(0):
            pass

        for b in range(B):
            # Filler queues: proj-ish (deferred q2/q3 of batch b, then
            # proj(b+1)) and wo rows of batch b (gated on sc completion).
            # Strict FIFO within proj-ish (q23 before proj(b+1)): proj(b+1)'s
            # xt DMA reuses q23(b)'s xt slot, so emitting it before all
            # q23(b) matmuls are on the PE queue would deadlock the in-order
            # queues.  Between classes we ALTERNATE: consecutive proj groups
            # otherwise stall ~3us each on the "qk" PSUM slot, whose freeing
            # ACT copy sits behind the exp stream (measured via NTFF).
            pf = [projq23_batch(b)]
            if b + 1 < B:
                pf.append(proj_batch(b + 1))
            wf = []
            rr = [0]

            def drain_filler(n, pf=pf, wf=wf, rr=rr):
                done = 0
                while done < n and (pf or wf):
                    use_wf = bool(wf) and (not pf or rr[0] % 2 == 1)
                    rr[0] += 1
                    gen = wf[0] if use_wf else pf[0]
                    try:
                        next(gen)
                        done += 1
                    except StopIteration:
                        (wf if use_wf else pf).remove(gen)
                return done

            if do_attn:
                for step in attn_batch(b):
                    if isinstance(step, tuple) and step[0] == "sc_done":
                        sc = step[1]
                        if do_wo:
                            tts = range(sc * (SCW // 128),
                                        (sc + 1) * (SCW // 128))
                            alt = (b == B - 1) or (sc == S // SCW - 1)
                            wf.append(wo_rows(b, tts, alt))
                        continue
                    # one attn pair-step emitted (4 matmuls); ~2 filler
                    # steps.  The last batch has only ~84 filler steps for
                    # 72 yields — drain 1 so they stretch across the batch.
                    drain_filler(2 if b + 1 < B else 1)
            elif do_wo:
                wf.append(wo_rows(b, range(S // 128), alt=True))
            # drain all remaining wo(b) (+ any proj(b+1) leftovers)
            while drain_filler(64):
                pass


def _rope_perm():
    # even features first, then odd — per 128-wide head
    return np.concatenate([np.arange(0, HD, 2), np.arange(1, HD, 2)])


def _prep_inputs(x, freqs_cos, freqs_sin, cache_k, cache_v, wq, wk, wv, wo):
    perm = _rope_perm()
    xt = np.ascontiguousarray(
        x.reshape(T, DIM).T).astype(NP_BF16)
    cos_t = np.ascontiguousarray(freqs_cos.T).astype(NP_BF16)
    sin_t = np.ascontiguousarray(freqs_sin.T).astype(NP_BF16)

    in_maps = []
    for m in range(NCORES):
        wq_m = wq[m * NQ * HD:(m + 1) * NQ * HD]  # (512, 4096)
        wq_m = wq_m.reshape(NQ, HD, DIM)[:, perm, :].reshape(NQ * HD, DIM)
        wqt_m = np.ascontiguousarray(wq_m.T).astype(NP_BF16)
        wk_m = wk[m * HD:(m + 1) * HD][perm]
        wkt_m = np.ascontiguousarray(wk_m.T).astype(NP_BF16)
        wv_m = wv[m * HD:(m + 1) * HD]
        wvt_m = np.ascontiguousarray(wv_m.T).astype(NP_BF16)
        wot_m = np.ascontiguousarray(
            wo[:, m * NQ * HD:(m + 1) * NQ * HD].T).astype(NP_BF16)
        ckt_m = np.ascontiguousarray(
            cache_k[:, m][:, :, perm].transpose(0, 2, 1)).astype(NP_BF16)
        cv_m = np.ascontiguousarray(cache_v[:, m]).astype(NP_BF16)
        in_maps.append({
            "xt": xt, "wqt": wqt_m, "wkt": wkt_m, "wvt": wvt_m,
            "wot": wot_m, "ckt": ckt_m, "cv": cv_m,
            "cos": cos_t, "sin": sin_t,
        })
    return in_maps


def kernel(x, freqs_cos, freqs_sin, cache_k, cache_v, wq, wk, wv, wo):
    global LAST_EXEC_NS, LAST_RESULTS
    if "nc" not in _CACHED:
        _CACHED["nc"] = _build_nc()
    nc = _CACHED["nc"]

    in_maps = _prep_inputs(np.asarray(x), np.asarray(freqs_cos),
                           np.asarray(freqs_sin), np.asarray(cache_k),
                           np.asarray(cache_v), np.asarray(wq),
                           np.asarray(wk), np.asarray(wv), np.asarray(wo))

    trace = os.environ.get("KERNEL_TRACE", "0") == "1"
    try:
        res = run_bass_kernel_spmd(nc, in_maps, core_ids=list(range(NCORES)),
                                   trace=trace)
    except (ImportError, ModuleNotFoundError):
        # NTFF profiling hook unavailable in this environment
        res = run_bass_kernel_spmd(nc, in_maps, core_ids=list(range(NCORES)),
                                   trace=False)
    LAST_EXEC_NS = res.exec_time_ns
    LAST_RESULTS = res

    total = np.zeros((T, DIM), dtype=np.float32)
    for r in res.results:
        total += r["out_p"].astype(np.float32)
    return total.reshape(B, S, DIM)



# revision 40
# speedup vs baseline: 1.0050x; 1.0050x over previous
"""Multi-head attention (GQA prefill with KV cache) on 8 trn2 NeuronCores.

Sharding: tensor-parallel over heads. Core m owns KV head m (of 8) and the
4 query heads 4m..4m+3.  Each core computes its heads' attention output and
a partial x @ wo.T contribution; the host sums the 8 partials.

Layout notes:
  - All activations on device are "feature-major" ([feature, token]) so the
    token dim rides the matmul moving dim; x is transposed on the host once.
  - RoPE's (even, odd) pair interleave is removed by permuting wq/wk rows and
    cache_k's head_dim on the host (QK^T is invariant to a shared permutation
    of head_dim), so on device RoPE is plain 64-partition block arithmetic.
  - Scores are computed transposed ([key, query]) so the softmax denominator
    is an all-ones matmul and the attention output lands feature-major,
    which feeds the wo matmul directly.

Pipeline notes (v2):
  - PSUM: four independent 1-bank tags (qk / sc / out / wo), bufs=2 each,
    so the three phases never serialize on pool rotation.
  - Softmax denominators accumulate on the (otherwise idle) DVE in fp16;
    a single all-ones [128,128] matmul broadcasts the per-query sums to all
    partitions, replacing 16 ones-vector matmuls per attention unit.
  - Emission is software-pipelined at sub-unit granularity: projection and
    wo-projection matmul groups are interleaved between attention kt-steps,
    keeping PE busy while the exp stream (ACT-paced) advances.
  - Output partials are written bf16, one DMA per 128-token row.
"""

import os
import sys

import numpy as np

if "/opt/trn_rl_repo" not in sys.path:
    sys.path.insert(0, "/opt/trn_rl_repo")

import ml_dtypes

import concourse.bass as bass
import concourse.mybir as mybir
import concourse.tile as tile
from concourse.bass_utils import run_bass_kernel_spmd
from concourse.masks import make_identity

BF16 = mybir.dt.bfloat16
FP16 = mybir.dt.float16
F32 = mybir.dt.float32
NP_BF16 = ml_dtypes.bfloat16

B, S, DIM = 4, 1024, 4096
N_HEADS, N_KV_HEADS = 32, 8
HD = 128
PAST = 1024
NCORES = 8
NQ = N_HEADS // NCORES  # 4 q heads per core
T = B * S  # 4096 tokens
DT = DIM // 128  # 32 contraction tiles
CH = 512  # projection token-chunk
NCH_B = S // CH  # chunks per batch (2)
SCW = 512  # attention s-chunk width
NKT = (PAST + S) // 128  # 16 key tiles per batch
ISQRT_HD = 1.0 / float(np.sqrt(HD))

LAST_EXEC_NS = None
LAST_RESULTS = None

_CACHED = {}


def _split_multi_waits(nc):
    """walrus' per-instruction sync encoding fits one wait; hoist extras
    onto standalone EventSemaphore instructions on the same engine queue."""
    for f in nc.m.functions:
        for blk in f.blocks:
            insts = blk.instructions
            if not any(i.sync_info and i.sync_info.on_wait
                       and len(i.sync_info.on_wait) > 1 for i in insts):
                continue
            new = []
            for inst in insts:
                si = inst.sync_info
                if si is not None and si.on_wait and len(si.on_wait) > 1:
                    waits = list(si.on_wait)
                    for wt in waits[:-1]:
                        evs = mybir.InstEventSemaphore(
                            name=f"I-wsplit-{nc.next_id()}", ins=[], outs=[])
                        evs.engine = inst.engine
                        evs.sync_info = mybir.SyncInfo(on_wait=[wt],
                                                       on_update=[])
                        new.append(evs)
                    inst.sync_info = mybir.SyncInfo(
                        on_wait=[waits[-1]],
                        on_update=list(si.on_update or []))
                new.append(inst)
            insts[:] = new


def _build_nc(split_waits=True, mode="full", reps=1):
    nc = bass.Bass("TRN2", target_bir_lowering=False, debug=False,
                   num_devices=NCORES)

    xt = nc.dram_tensor("xt", [DIM, T], BF16, kind="ExternalInput")
    wqt = nc.dram_tensor("wqt", [DIM, NQ * HD], BF16, kind="ExternalInput")
    wkt = nc.dram_tensor("wkt", [DIM, HD], BF16, kind="ExternalInput")
    wvt = nc.dram_tensor("wvt", [DIM, HD], BF16, kind="ExternalInput")
    wot = nc.dram_tensor("wot", [NQ * HD, DIM], BF16, kind="ExternalInput")
    ckt = nc.dram_tensor("ckt", [B, HD, PAST], BF16, kind="ExternalInput")
    cv = nc.dram_tensor("cv", [B, PAST, HD], BF16, kind="ExternalInput")
    cos = nc.dram_tensor("cos", [HD // 2, S], BF16, kind="ExternalInput")
    sin = nc.dram_tensor("sin", [HD // 2, S], BF16, kind="ExternalInput")
    out_p = nc.dram_tensor("out_p", [T, DIM], BF16, kind="ExternalOutput")

    with tile.TileContext(nc) as tc:
        if mode == "null":
            _emit_null(tc, nc)
        else:
            for _ in range(reps):
                _emit(tc, nc, xt, wqt, wkt, wvt, wot, ckt, cv, cos, sin,
                      out_p, mode=mode)
    if split_waits:
        _split_multi_waits(nc)
    return nc


def _emit_null(tc, nc):
    """Near-empty program: measures per-dispatch overhead in bench.py."""
    from contextlib import ExitStack
    with ExitStack() as ctx:
        cw = ctx.enter_context(tc.tile_pool(name="nullp", bufs=1))
        t = cw.tile([128, 128], BF16, name="null_t")
        nc.vector.memset(t, 0.0)


def _emit(tc, nc, xt, wqt, wkt, wvt, wot, ckt, cv, cos, sin, out_p,
          mode="full"):
    from contextlib import ExitStack
    do_attn = mode in ("full", "bc")
    do_wo = mode in ("full", "bd")

    with ExitStack() as ctx:
        cw = ctx.enter_context(tc.tile_pool(name="consts", bufs=1))
        pb = ctx.enter_context(tc.tile_pool(name="perbatch", bufs=2))
        wk = ctx.enter_context(tc.tile_pool(name="work", bufs=2))
        ps = ctx.enter_context(tc.tile_pool(name="ps", bufs=2, space="PSUM"))

        # ---- resident constants -------------------------------------------
        # DMA order is startup-critical: the first emitted work is the k/v
        # projection of batch 0 chunk 0, which needs only wkt/wvt (+ the xt
        # chunk, issued inside proj_batch).  wqt follows (needed ~15us later
        # by the first q unit); wot and the kv-cache much later.
        wkt_sb = cw.tile([128, DT * HD], BF16, name="wkt_sb")
        nc.sync.dma_start(
            out=wkt_sb.rearrange("p (n j) -> p n j", n=DT),
            in_=wkt[:, :].rearrange("(n p) j -> p n j", p=128))
        wvt_sb = cw.tile([128, DT * HD], BF16, name="wvt_sb")

        def load_wvt():
            nc.sync.dma_start(
                out=wvt_sb.rearrange("p (n j) -> p n j", n=DT),
                in_=wvt[:, :].rearrange("(n p) j -> p n j", p=128))
        # cos/sin duplicated across both 64-partition halves so RoPE's two
        # multiplies can run full-width: q*cos gives (r*cos | i*cos).
        # (loaded after the first xt chunk — see proj_batch)
        cos_sb = cw.tile([128, S], BF16, name="cos_sb")
        sin_sb = cw.tile([128, S], BF16, name="sin_sb")

        def load_cos_sin():
            nc.sync.dma_start(out=cos_sb[0:64, :], in_=cos[:, :])
            nc.sync.dma_start(out=cos_sb[64:128, :], in_=cos[:, :])
            nc.sync.dma_start(out=sin_sb[0:64, :], in_=sin[:, :])
            nc.sync.dma_start(out=sin_sb[64:128, :], in_=sin[:, :])

        ones128 = cw.tile([128, 128], FP16, name="ones128")
        nc.vector.memset(ones128, 1.0)
        ident = cw.tile([128, 128], BF16, name="ident")
        make_identity(nc, ident)
        wqt_sb = cw.tile([128, DT * NQ * HD], BF16, name="wqt_sb")
        wot_sb = cw.tile([128, NQ * DIM], BF16, name="wot_sb")

        def load_wqt():
            nc.sync.dma_start(
                out=wqt_sb.rearrange("p (n j) -> p n j", n=DT),
                in_=wqt[:, :].rearrange("(n p) j -> p n j", p=128))

        def load_wot():
            nc.sync.dma_start(
                out=wot_sb.rearrange("p (n d) -> p n d", n=NQ),
                in_=wot[:, :].rearrange("(n p) d -> p n d", p=128))

        # ---- per-batch / work tiles (slot handles, rotated via tags) ------
        def batch_tiles(b):
            qb_t = pb.tile([128, NQ * S], BF16, name="qb", tag="qb")
            kb_t = pb.tile([128, S], BF16, name="kb", tag="kb")
            vb_t = pb.tile([128, S], BF16, name="vb", tag="vb")
            attnb_t = pb.tile([128, NQ * S], BF16, name="attnb", tag="attnb",
                              bufs=1)
            ckt_b = pb.tile([128, PAST], BF16, name="ckt_b", tag="ckt_b")
            cv_b = pb.tile([128, PAST], BF16, name="cv_b", tag="cv_b")
            return dict(qb=qb_t, kb=kb_t, vb=vb_t, attnb=attnb_t,
                        ckt=ckt_b, cv=cv_b)

        def rope(dst_tile, dst_col, src_ps, cosc, sinc, n):
            """src layout (r|i) on partition halves.
            dst[0:64] = r*cos - i*sin ; dst[64:128] = r*sin + i*cos.
            A fast ACT copy stages src to fp16 SBUF first, freeing the "qk"
            PSUM slot immediately (the DVE rope ops otherwise queue behind
            attention acc-adds and stall the next proj group's matmuls).
            Two full-width muls: tc = (r*cos | i*cos), ts = (r*sin | i*sin),
            then dst_r = tc_hi - ts_lo, dst_i = ts_hi + tc_lo."""
            qraw = wk.tile([128, CH], FP16, name="qraw", tag="qraw", bufs=2)
            nc.scalar.copy(qraw[:, :n], src_ps)
            tc_ = wk.tile([128, CH], FP16, name="rope_tc", tag="rope_tc",
                          bufs=1)
            ts_ = wk.tile([128, CH], FP16, name="rope_ts", tag="rope_ts",
                          bufs=1)
            nc.vector.tensor_mul(tc_[:, :n], qraw[:, :n], cosc)
            nc.vector.tensor_mul(ts_[0:64, :n], qraw[64:128, :n],
                                 sinc[64:128, :])
            nc.vector.tensor_mul(ts_[64:128, :n], qraw[0:64, :n],
                                 sinc[0:64, :])
            nc.vector.tensor_sub(dst_tile[0:64, dst_col:dst_col + n],
                                 tc_[0:64, :n], ts_[0:64, :n])
            nc.vector.tensor_add(dst_tile[64:128, dst_col:dst_col + n],
                                 ts_[64:128, :n], tc_[64:128, :n])

        # ================= work-unit generators (each yields steps) ========
        def xt_load(b, c, after_piece=None):
            """DMA one 512-token chunk of x into SBUF (feature-major).
            Split so the projection's first accum matmuls can start as soon
            as the first piece lands (subtile deps).  after_piece maps piece
            index -> load callbacks interleaved onto the (serial) DMA ring
            by the time their consumer actually needs them."""
            xt_t = wk.tile([128, DT * CH], BF16, name="xt_t", tag="xt")
            p0 = c * CH
            # First chunk is latency-critical (PE idles until d0.. land):
            # progressive split so the first accum matmuls start ~7us in.
            splits = [2, 2, 4, 8, 16] if (b == 0 and c == 0) else [16, 16]
            d0 = 0
            for pi, step in enumerate(splits):
                nc.sync.dma_start(
                    out=xt_t[:, d0 * CH:(d0 + step) * CH].rearrange(
                        "p (n t) -> p n t", n=step),
                    in_=xt[128 * d0:128 * (d0 + step),
                           b * S + p0: b * S + p0 + CH].rearrange(
                        "(n p) t -> p n t", p=128))
                d0 += step
                for cb in (after_piece or {}).get(pi, ()):
                    cb()
            return xt_t

        def proj_unit(bt, b, c, kind, xt_t, alt=False):
            """One projection group: 32 accum matmuls + rope (or v-transpose).
            kind: 0..3 = q head j, 4 = k, 5 = v. Yields 5 steps.
            alt: prologue groups run back-to-back with no filler, so they
            alternate onto the (then idle) "out" bank instead of
            serializing on the single "qk" bank."""
            p0 = c * CH
            cosc = cos_sb[:, p0:p0 + CH]
            sinc = sin_sb[:, p0:p0 + CH]
            tag = "out" if (alt and kind % 2 == 1) else "qk"
            acc_ps = ps.tile([128, CH], F32, name="proj_ps", tag=tag,
                             bufs=2 if tag == "out" else 1)
            if kind < NQ:
                w_sb, wof = wqt_sb, kind * HD
                wstride = NQ * HD
            elif kind == NQ:
                w_sb, wof, wstride = wkt_sb, 0, HD
            else:
                w_sb, wof, wstride = wvt_sb, 0, HD
            for d0 in range(0, DT, 8):
                for d in range(d0, d0 + 8):
                    nc.tensor.matmul(
                        acc_ps,
                        lhsT=w_sb[:, d * wstride + wof:
                                  d * wstride + wof + HD],
                        rhs=xt_t[:, d * CH:(d + 1) * CH],
                        start=(d == 0), stop=(d == DT - 1))
                yield  # ~8 x 213ns PE quantum
            if kind < NQ:
                rope(bt["qb"], kind * S + p0, acc_ps, cosc, sinc, CH)
            elif kind == NQ:
                rope(bt["kb"], p0, acc_ps, cosc, sinc, CH)
            else:
                vcp = wk.tile([128, CH], BF16, name="vcp", tag="vcp", bufs=1)
                nc.scalar.copy(vcp, acc_ps)
                for tsub in range(CH // 128):
                    vtr_ps = ps.tile([128, 128], BF16, name="vtr_ps",
                                     tag="sc")
                    nc.tensor.transpose(vtr_ps,
                                        vcp[:, tsub * 128:(tsub + 1) * 128],
                                        ident)
                    col = (c * (CH // 128) + tsub) * 128
                    nc.vector.tensor_copy(bt["vb"][:, col:col + 128], vtr_ps)
            yield

        def attn_unit(bt, b, sc, h, epi_out):
            """Attention for (query-chunk sc, head h): 16 kt steps.
            Scores land [key, query] in PSUM; exp on ACT; denominator
            accumulates on DVE in fp16; all-ones matmul broadcasts sums.

            Pipelining (v3, NTFF-driven):
              - av(kt) is emitted one step behind scores(kt+1), so the PE
                never waits on the ACT exp stream mid-unit.
              - The epilogue (sums broadcast + reciprocal + normalize) is
                handed back via epi_out and emitted ~2 kt-steps into the
                NEXT unit, so its ACT copy does not delay exp(kt0') and the
                sums matmuls get runway before reading the DVE accs."""
            s0 = sc * SCW
            out_ps = ps.tile([128, SCW], F32, name="out_ps", tag="out")
            acc0 = wk.tile([128, SCW], FP16, name="acc0", tag="acc0", bufs=1)
            acc1 = wk.tile([128, SCW], FP16, name="acc1", tag="acc1", bufs=1)
            qsl = bt["qb"][:, h * S + s0:h * S + s0 + SCW]

            def kv(kt):
                if kt < PAST // 128:
                    return (bt["ckt"][:, kt * 128:(kt + 1) * 128],
                            bt["cv"][:, kt * 128:(kt + 1) * 128])
                kn = kt - PAST // 128
                return (bt["kb"][:, kn * 128:(kn + 1) * 128],
                        bt["vb"][:, kn * 128:(kn + 1) * 128])

            # kt-steps processed in PAIRS sharing one 2-bank PSUM tile so a
            # single [128,1024] exp covers both — halves ACT instruction
            # count/overhead (the exp stream otherwise paces attention).
            prev = None
            for p in range(NKT // 2):
                k0, v0 = kv(2 * p)
                k1, v1 = kv(2 * p + 1)
                sc_ps = ps.tile([128, 2 * SCW], F32, name="sc_ps", tag="sc")
                nc.tensor.matmul(sc_ps[:, 0:SCW], lhsT=k0, rhs=qsl)
                nc.tensor.matmul(sc_ps[:, SCW:2 * SCW], lhsT=k1, rhs=qsl)
                if prev is not None:
                    pexp, pv0, pv1 = prev
                    nc.tensor.matmul(out_ps, lhsT=pv0, rhs=pexp[:, 0:SCW],
                                     start=(p == 1), stop=False)
                    nc.tensor.matmul(out_ps, lhsT=pv1,
                                     rhs=pexp[:, SCW:2 * SCW],
                                     start=False, stop=False)
                exp_t = wk.tile([128, 2 * SCW], BF16, name="exp_t",
                                tag="exp", bufs=2)
                nc.scalar.activation(exp_t, sc_ps,
                                     mybir.ActivationFunctionType.Exp,
                                     scale=ISQRT_HD)
                if p == 0:
                    nc.vector.tensor_copy(acc0, exp_t[:, 0:SCW])
                    nc.vector.tensor_copy(acc1, exp_t[:, SCW:2 * SCW])
                else:
                    nc.vector.tensor_add(acc0, acc0, exp_t[:, 0:SCW])
                    nc.vector.tensor_add(acc1, acc1, exp_t[:, SCW:2 * SCW])
                prev = (exp_t, v0, v1)
                yield
            pexp, pv0, pv1 = prev
            nc.tensor.matmul(out_ps, lhsT=pv0, rhs=pexp[:, 0:SCW],
                             start=False, stop=False)
            nc.tensor.matmul(out_ps, lhsT=pv1, rhs=pexp[:, SCW:2 * SCW],
                             start=False, stop=True)

            def epilogue():
                sums_ps = ps.tile([128, SCW], F32, name="sums_ps", tag="sc")
                nc.tensor.matmul(sums_ps, lhsT=ones128, rhs=acc0,
                                 start=True, stop=False)
                nc.tensor.matmul(sums_ps, lhsT=ones128, rhs=acc1,
                                 start=False, stop=True)
                # Fast ACT copy frees the "sc" PSUM slot quickly; the
                # reciprocal (3.4us on real DVE) runs in-place afterwards,
                # off the PE-critical path — quartered so the first attnb
                # 128-token tile lands ~3us sooner (wo rows consume attnb
                # per 128-token tile and otherwise stall on batch 3's tail).
                inv_bc = wk.tile([128, SCW], BF16, name="inv_bc", tag="inv",
                                 bufs=1)
                nc.scalar.copy(inv_bc, sums_ps)
                for qtr in range(SCW // 128):
                    sl = slice(qtr * 128, (qtr + 1) * 128)
                    with nc.allow_low_precision(reason="softmax denom recip"):
                        nc.vector.reciprocal(inv_bc[:, sl], inv_bc[:, sl])
                    nc.vector.tensor_mul(
                        bt["attnb"][:, h * S + s0 + qtr * 128:
                                    h * S + s0 + (qtr + 1) * 128],
                        out_ps[:, sl], inv_bc[:, sl])
            epi_out.append(epilogue)
            yield

        NDC = DIM // SCW  # 8 wo column-groups per token tile
        HROW = NDC // 4   # groups per staging tile (quarter row)

        def wo_unit(bt, b, tt, dc, st_half, alt):
            """Partial x@wo.T for token-tile tt, 512 output dims dc.
            alt: when the proj fillers can no longer touch "qk" (all of the
            last batch, each batch's final chunk), odd groups borrow that
            idle bank so back-to-back wo groups don't serialize on the
            single "wo" bank waiting for the staging copy."""
            tag = "qk" if (alt and dc % 2 == 1) else "wo"
            wo_ps = ps.tile([128, SCW], F32, name="wo_ps", tag=tag,
                            bufs=1)
            for j in range(NQ):
                nc.tensor.matmul(
                    wo_ps,
                    lhsT=bt["attnb"][:, j * S + tt * 128:
                                     j * S + (tt + 1) * 128],
                    rhs=wot_sb[:, j * DIM + dc * SCW:
                               j * DIM + (dc + 1) * SCW],
                    start=(j == 0), stop=(j == NQ - 1))
            # GPSIMD cannot read PSUM.  All staging copies go to ACT: with
            # paired exps ACT is ~45% loaded, while the DVE queue carries
            # the epilogue reciprocal bursts that would delay the copy and
            # stall the single-banked wo_ps slot (measured via NTFF).
            dst = st_half[:, (dc % HROW) * SCW:(dc % HROW + 1) * SCW]
            nc.scalar.copy(dst, wo_ps)
            if dc % HROW == HROW - 1:
                row = (b * (S // 128) + tt) * 128
                col = (dc // HROW) * HROW * SCW
                nc.sync.dma_start(
                    out=out_p[row:row + 128, col:col + HROW * SCW],
                    in_=st_half)
            yield

        def wo_row(bt, b, tt, alt):
            for part in range(NDC // HROW):
                st_half = wk.tile([128, HROW * SCW], BF16, name="st_half",
                                  tag="st", bufs=2)
                for dc in range(part * HROW, (part + 1) * HROW):
                    yield from wo_unit(bt, b, tt, dc, st_half, alt)

        # ================= scheduler ======================================
        # Per batch: proj(b) is fully emitted before attn(b).  attn(b)'s 8
        # units are ACT-paced; between kt-steps we interleave filler steps
        # from wo(b) rows (ready once all heads of their s-chunk are done)
        # and proj(b+1) (+ its xt prefetch).  All wo(b) rows are emitted
        # before attn(b+1) starts so attnb can be single-buffered.
        bts = [None] * B
        xts = [None] * B

        def cache_load(b):
            bt = bts[b]
            nc.sync.dma_start(out=bt["ckt"], in_=ckt[b, :, :])
            nc.sync.dma_start(
                out=bt["cv"].rearrange("p (n d) -> p n d", n=PAST // 128),
                in_=cv[b, :, :].rearrange("(n p) t -> p n t", p=128))

        def proj_batch(b):
            """Generator over proj(b) steps (incl. xt loads + cache DMA).
            The kv-cache DMA is issued after the first xt chunk so it does
            not delay the projection-critical loads on the in-order queue.
            Heads q2/q3 are deferred (see projq23_batch): they are only
            needed by attention units h2/h3, so they serve as attn-phase
            filler — which matters for the last batch, whose attention has
            no next-batch projections to interleave."""
            bts[b] = batch_tiles(b)
            xts[b] = [None] * NCH_B
            for c in range(NCH_B):
                if b == 0 and c == 0:
                    # interleave weight/trig loads between xt pieces by
                    # consumer need-time: cos/sin before the k-rope (~20us),
                    # wvt before the v unit (~25us), wqt before q0 (~36us).
                    xts[b][c] = xt_load(b, c, after_piece={
                        1: [load_cos_sin, load_wvt],
                        2: [load_wqt]})
                else:
                    xts[b][c] = xt_load(b, c)
                if b == 0:
                    if c == 1:
                        load_wot()
                        cache_load(b)
                elif c == 0:
                    cache_load(b)
                for kind in (NQ, NQ + 1, 0, 1):  # k, v first
                    yield from proj_unit(bts[b], b, c, kind, xts[b][c],
                                         alt=(b == 0))

        def projq23_batch(b):
            for c in range(NCH_B):
                for kind in (2, 3):
                    yield from proj_unit(bts[b], b, c, kind, xts[b][c])

        def attn_batch(b):
            epis = []        # deferred epilogue of the previous unit
            pending_sc = None  # sc-chunk whose last epilogue is deferred
            for sc in range(S // SCW):
                for h in range(NQ if do_attn else 0):
                    step_n = 0
                    for _ in attn_unit(bts[b], b, sc, h, epis):
                        yield
                        step_n += 1
                        if step_n == 2 and epis:
                            epis.pop(0)()
                            if pending_sc is not None:
                                yield ("sc_done", pending_sc)
                                pending_sc = None
                pending_sc = sc
            while epis:
                epis.pop(0)()
            if pending_sc is not None:
                yield ("sc_done", pending_sc)

        def wo_rows(b, tts, alt=False):
            for tt in tts:
                yield from wo_row(bts[b], b, tt, alt)

        # prologue: batch 0 projections (k/v/q0/q1), no filler available
        for _ in proj_batch(0):
            pass

        for b in range(B):
            # Filler queues: proj-ish (deferred q2/q3 of batch b, then
            # proj(b+1)) and wo rows of batch b (gated on sc completion).
            # Strict FIFO within proj-ish (q23 before proj(b+1)): proj(b+1)'s
            # xt DMA reuses q23(b)'s xt slot, so emitting it before all
            # q23(b) matmuls are on the PE queue would deadlock the in-order
            # queues.  Between classes we ALTERNATE: consecutive proj groups
            # otherwise stall ~3us each on the "qk" PSUM slot, whose freeing
            # ACT copy sits behind the exp stream (measured via NTFF).
            pf = [projq23_batch(b)]
            if b + 1 < B:
                pf.append(proj_batch(b + 1))
            wf = []
            rr = [0]

            def drain_filler(n, pf=pf, wf=wf, rr=rr):
                done = 0
                while done < n and (pf or wf):
                    use_wf = bool(wf) and (not pf or rr[0] % 2 == 1)
                    rr[0] += 1
                    gen = wf[0] if use_wf else pf[0]
                    try:
                        next(gen)
                        done += 1
                    except StopIteration:
                        (wf if use_wf else pf).remove(gen)
                return done

            if do_attn:
                for step in attn_batch(b):
                    if isinstance(step, tuple) and step[0] == "sc_done":
                        sc = step[1]
                        if do_wo:
                            tts = range(sc * (SCW // 128),
                                        (sc + 1) * (SCW // 128))
                            alt = (b == B - 1) or (sc == S // SCW - 1)
                            wf.append(wo_rows(b, tts, alt))
                        continue
                    # one attn pair-step emitted (4 matmuls); ~2 filler
                    # steps.  The last batch has only ~84 filler steps for
                    # 72 yields — drain 1 so they stretch across the batch.
                    drain_filler(2 if b + 1 < B else 1)
            elif do_wo:
                wf.append(wo_rows(b, range(S // 128), alt=True))
            # drain all remaining wo(b) (+ any proj(b+1) leftovers)
            while drain_filler(64):
                pass


def _rope_perm():
    # even features first, then odd — per 128-wide head
    return np.concatenate([np.arange(0, HD, 2), np.arange(1, HD, 2)])


def _prep_inputs(x, freqs_cos, freqs_sin, cache_k, cache_v, wq, wk, wv, wo):
    perm = _rope_perm()
    xt = np.ascontiguousarray(
        x.reshape(T, DIM).T).astype(NP_BF16)
    cos_t = np.ascontiguousarray(freqs_cos.T).astype(NP_BF16)
    sin_t = np.ascontiguousarray(freqs_sin.T).astype(NP_BF16)

    in_maps = []
    for m in range(NCORES):
        wq_m = wq[m * NQ * HD:(m + 1) * NQ * HD]  # (512, 4096)
        wq_m = wq_m.reshape(NQ, HD, DIM)[:, perm, :].reshape(NQ * HD, DIM)
        wqt_m = np.ascontiguousarray(wq_m.T).astype(NP_BF16)
        wk_m = wk[m * HD:(m + 1) * HD][perm]
        wkt_m = np.ascontiguousarray(wk_m.T).astype(NP_BF16)
        wv_m = wv[m * HD:(m + 1) * HD]
        wvt_m = np.ascontiguousarray(wv_m.T).astype(NP_BF16)
        wot_m = np.ascontiguousarray(
            wo[:, m * NQ * HD:(m + 1) * NQ * HD].T).astype(NP_BF16)
        ckt_m = np.ascontiguousarray(
            cache_k[:, m][:, :, perm].transpose(0, 2, 1)).astype(NP_BF16)
        cv_m = np.ascontiguousarray(cache_v[:, m]).astype(NP_BF16)
        in_maps.append({
            "xt": xt, "wqt": wqt_m, "wkt": wkt_m, "wvt": wvt_m,
            "wot": wot_m, "ckt": ckt_m, "cv": cv_m,
            "cos": cos_t, "sin": sin_t,
        })
    return in_maps


def kernel(x, freqs_cos, freqs_sin, cache_k, cache_v, wq, wk, wv, wo):
    global LAST_EXEC_NS, LAST_RESULTS
    if "nc" not in _CACHED:
        _CACHED["nc"] = _build_nc()
    nc = _CACHED["nc"]

    in_maps = _prep_inputs(np.asarray(x), np.asarray(freqs_cos),
                           np.asarray(freqs_sin), np.asarray(cache_k),
                           np.asarray(cache_v), np.asarray(wq),
                           np.asarray(wk), np.asarray(wv), np.asarray(wo))

    trace = os.environ.get("KERNEL_TRACE", "0") == "1"
    try:
        res = run_bass_kernel_spmd(nc, in_maps, core_ids=list(range(NCORES)),
                                   trace=trace)
    except (ImportError, ModuleNotFoundError):
        # NTFF profiling hook unavailable in this environment
        res = run_bass_kernel_spmd(nc, in_maps, core_ids=list(range(NCORES)),
                                   trace=False)
    LAST_EXEC_NS = res.exec_time_ns
    LAST_RESULTS = res

    total = np.zeros((T, DIM), dtype=np.float32)
    for r in res.results:
        total += r["out_p"].astype(np.float32)
    return total.reshape(B, S, DIM)



# revision 44
# speedup vs baseline: 1.0093x; 1.0043x over previous
"""Multi-head attention (GQA prefill with KV cache) on 8 trn2 NeuronCores.

Sharding: tensor-parallel over heads. Core m owns KV head m (of 8) and the
4 query heads 4m..4m+3.  Each core computes its heads' attention output and
a partial x @ wo.T contribution; the host sums the 8 partials.

Layout notes:
  - All activations on device are "feature-major" ([feature, token]) so the
    token dim rides the matmul moving dim; x is transposed on the host once.
  - RoPE's (even, odd) pair interleave is removed by permuting wq/wk rows and
    cache_k's head_dim on the host (QK^T is invariant to a shared permutation
    of head_dim), so on device RoPE is plain 64-partition block arithmetic.
  - Scores are computed transposed ([key, query]) so the softmax denominator
    is an all-ones matmul and the attention output lands feature-major,
    which feeds the wo matmul directly.

Pipeline notes (v2):
  - PSUM: four independent 1-bank tags (qk / sc / out / wo), bufs=2 each,
    so the three phases never serialize on pool rotation.
  - Softmax denominators accumulate on the (otherwise idle) DVE in fp16;
    a single all-ones [128,128] matmul broadcasts the per-query sums to all
    partitions, replacing 16 ones-vector matmuls per attention unit.
  - Emission is software-pipelined at sub-unit granularity: projection and
    wo-projection matmul groups are interleaved between attention kt-steps,
    keeping PE busy while the exp stream (ACT-paced) advances.
  - Output partials are written bf16, one DMA per 128-token row.
"""

import os
import sys

import numpy as np

if "/opt/trn_rl_repo" not in sys.path:
    sys.path.insert(0, "/opt/trn_rl_repo")

import ml_dtypes

import concourse.bass as bass
import concourse.mybir as mybir
import concourse.tile as tile
from concourse.bass_utils import run_bass_kernel_spmd
from concourse.masks import make_identity

BF16 = mybir.dt.bfloat16
FP16 = mybir.dt.float16
F32 = mybir.dt.float32
NP_BF16 = ml_dtypes.bfloat16

B, S, DIM = 4, 1024, 4096
N_HEADS, N_KV_HEADS = 32, 8
HD = 128
PAST = 1024
NCORES = 8
NQ = N_HEADS // NCORES  # 4 q heads per core
T = B * S  # 4096 tokens
DT = DIM // 128  # 32 contraction tiles
CH = 512  # projection token-chunk
NCH_B = S // CH  # chunks per batch (2)
SCW = 512  # attention s-chunk width
NKT = (PAST + S) // 128  # 16 key tiles per batch
ISQRT_HD = 1.0 / float(np.sqrt(HD))

LAST_EXEC_NS = None
LAST_RESULTS = None

_CACHED = {}


def _split_multi_waits(nc):
    """walrus' per-instruction sync encoding fits one wait; hoist extras
    onto standalone EventSemaphore instructions on the same engine queue."""
    for f in nc.m.functions:
        for blk in f.blocks:
            insts = blk.instructions
            if not any(i.sync_info and i.sync_info.on_wait
                       and len(i.sync_info.on_wait) > 1 for i in insts):
                continue
            new = []
            for inst in insts:
                si = inst.sync_info
                if si is not None and si.on_wait and len(si.on_wait) > 1:
                    waits = list(si.on_wait)
                    for wt in waits[:-1]:
                        evs = mybir.InstEventSemaphore(
                            name=f"I-wsplit-{nc.next_id()}", ins=[], outs=[])
                        evs.engine = inst.engine
                        evs.sync_info = mybir.SyncInfo(on_wait=[wt],
                                                       on_update=[])
                        new.append(evs)
                    inst.sync_info = mybir.SyncInfo(
                        on_wait=[waits[-1]],
                        on_update=list(si.on_update or []))
                new.append(inst)
            insts[:] = new


def _build_nc(split_waits=True, mode="full", reps=1):
    nc = bass.Bass("TRN2", target_bir_lowering=False, debug=False,
                   num_devices=NCORES)

    xt = nc.dram_tensor("xt", [DIM, T], BF16, kind="ExternalInput")
    wqt = nc.dram_tensor("wqt", [DIM, NQ * HD], BF16, kind="ExternalInput")
    wkt = nc.dram_tensor("wkt", [DIM, HD], BF16, kind="ExternalInput")
    wvt = nc.dram_tensor("wvt", [DIM, HD], BF16, kind="ExternalInput")
    wot = nc.dram_tensor("wot", [NQ * HD, DIM], BF16, kind="ExternalInput")
    ckt = nc.dram_tensor("ckt", [B, HD, PAST], BF16, kind="ExternalInput")
    cv = nc.dram_tensor("cv", [B, PAST, HD], BF16, kind="ExternalInput")
    cos = nc.dram_tensor("cos", [HD // 2, S], BF16, kind="ExternalInput")
    sin = nc.dram_tensor("sin", [HD // 2, S], BF16, kind="ExternalInput")
    out_p = nc.dram_tensor("out_p", [T, DIM], BF16, kind="ExternalOutput")

    with tile.TileContext(nc) as tc:
        if mode == "null":
            _emit_null(tc, nc)
        else:
            for _ in range(reps):
                _emit(tc, nc, xt, wqt, wkt, wvt, wot, ckt, cv, cos, sin,
                      out_p, mode=mode)
    if split_waits:
        _split_multi_waits(nc)
    return nc


def _emit_null(tc, nc):
    """Near-empty program: measures per-dispatch overhead in bench.py."""
    from contextlib import ExitStack
    with ExitStack() as ctx:
        cw = ctx.enter_context(tc.tile_pool(name="nullp", bufs=1))
        t = cw.tile([128, 128], BF16, name="null_t")
        nc.vector.memset(t, 0.0)


def _emit(tc, nc, xt, wqt, wkt, wvt, wot, ckt, cv, cos, sin, out_p,
          mode="full"):
    from contextlib import ExitStack
    do_attn = mode in ("full", "bc")
    do_wo = mode in ("full", "bd")

    with ExitStack() as ctx:
        cw = ctx.enter_context(tc.tile_pool(name="consts", bufs=1))
        pb = ctx.enter_context(tc.tile_pool(name="perbatch", bufs=2))
        wk = ctx.enter_context(tc.tile_pool(name="work", bufs=2))
        ps = ctx.enter_context(tc.tile_pool(name="ps", bufs=2, space="PSUM"))

        # ---- resident constants -------------------------------------------
        # DMA order is startup-critical: the first emitted work is the k/v
        # projection of batch 0 chunk 0, which needs only wkt/wvt (+ the xt
        # chunk, issued inside proj_batch).  wqt follows (needed ~15us later
        # by the first q unit); wot and the kv-cache much later.
        wkt_sb = cw.tile([128, DT * HD], BF16, name="wkt_sb")
        nc.sync.dma_start(
            out=wkt_sb.rearrange("p (n j) -> p n j", n=DT),
            in_=wkt[:, :].rearrange("(n p) j -> p n j", p=128))
        wvt_sb = cw.tile([128, DT * HD], BF16, name="wvt_sb")

        def load_wvt():
            nc.sync.dma_start(
                out=wvt_sb.rearrange("p (n j) -> p n j", n=DT),
                in_=wvt[:, :].rearrange("(n p) j -> p n j", p=128))
        # cos/sin duplicated across both 64-partition halves so RoPE's two
        # multiplies can run full-width: q*cos gives (r*cos | i*cos).
        # (loaded after the first xt chunk — see proj_batch)
        cos_sb = cw.tile([128, S], BF16, name="cos_sb")
        sin_sb = cw.tile([128, S], BF16, name="sin_sb")

        def load_cos_sin():
            nc.sync.dma_start(out=cos_sb[0:64, :], in_=cos[:, :])
            nc.sync.dma_start(out=cos_sb[64:128, :], in_=cos[:, :])
            nc.sync.dma_start(out=sin_sb[0:64, :], in_=sin[:, :])
            nc.sync.dma_start(out=sin_sb[64:128, :], in_=sin[:, :])

        ones128 = cw.tile([128, 128], FP16, name="ones128")
        nc.vector.memset(ones128, 1.0)
        ident = cw.tile([128, 128], BF16, name="ident")
        make_identity(nc, ident)
        wqt_sb = cw.tile([128, DT * NQ * HD], BF16, name="wqt_sb")
        wot_sb = cw.tile([128, NQ * DIM], BF16, name="wot_sb")

        def load_wqt():
            nc.sync.dma_start(
                out=wqt_sb.rearrange("p (n j) -> p n j", n=DT),
                in_=wqt[:, :].rearrange("(n p) j -> p n j", p=128))

        def load_wot():
            nc.sync.dma_start(
                out=wot_sb.rearrange("p (n d) -> p n d", n=NQ),
                in_=wot[:, :].rearrange("(n p) d -> p n d", p=128))

        # ---- per-batch / work tiles (slot handles, rotated via tags) ------
        def batch_tiles(b):
            qb_t = pb.tile([128, NQ * S], BF16, name="qb", tag="qb")
            kb_t = pb.tile([128, S], BF16, name="kb", tag="kb")
            vb_t = pb.tile([128, S], BF16, name="vb", tag="vb")
            attnb_t = pb.tile([128, NQ * S], BF16, name="attnb", tag="attnb",
                              bufs=1)
            ckt_b = pb.tile([128, PAST], BF16, name="ckt_b", tag="ckt_b")
            cv_b = pb.tile([128, PAST], BF16, name="cv_b", tag="cv_b")
            return dict(qb=qb_t, kb=kb_t, vb=vb_t, attnb=attnb_t,
                        ckt=ckt_b, cv=cv_b)

        def rope(dst_tile, dst_col, src_ps, cosc, sinc, n):
            """src layout (r|i) on partition halves.
            dst[0:64] = r*cos - i*sin ; dst[64:128] = r*sin + i*cos.
            A fast ACT copy stages src to fp16 SBUF first, freeing the "qk"
            PSUM slot immediately (the DVE rope ops otherwise queue behind
            attention acc-adds and stall the next proj group's matmuls).
            Two full-width muls: tc = (r*cos | i*cos), ts = (r*sin | i*sin),
            then dst_r = tc_hi - ts_lo, dst_i = ts_hi + tc_lo."""
            qraw = wk.tile([128, CH], FP16, name="qraw", tag="qraw", bufs=2)
            nc.scalar.copy(qraw[:, :n], src_ps)
            tc_ = wk.tile([128, CH], FP16, name="rope_tc", tag="rope_tc",
                          bufs=1)
            ts_ = wk.tile([128, CH], FP16, name="rope_ts", tag="rope_ts",
                          bufs=1)
            nc.vector.tensor_mul(tc_[:, :n], qraw[:, :n], cosc)
            nc.vector.tensor_mul(ts_[0:64, :n], qraw[64:128, :n],
                                 sinc[64:128, :])
            nc.vector.tensor_mul(ts_[64:128, :n], qraw[0:64, :n],
                                 sinc[0:64, :])
            nc.vector.tensor_sub(dst_tile[0:64, dst_col:dst_col + n],
                                 tc_[0:64, :n], ts_[0:64, :n])
            nc.vector.tensor_add(dst_tile[64:128, dst_col:dst_col + n],
                                 ts_[64:128, :n], tc_[64:128, :n])

        # ================= work-unit generators (each yields steps) ========
        def xt_load(b, c, after_piece=None):
            """DMA one 512-token chunk of x into SBUF (feature-major).
            Split so the projection's first accum matmuls can start as soon
            as the first piece lands (subtile deps).  after_piece maps piece
            index -> load callbacks interleaved onto the (serial) DMA ring
            by the time their consumer actually needs them."""
            xt_t = wk.tile([128, DT * CH], BF16, name="xt_t", tag="xt")
            p0 = c * CH
            # First chunk is latency-critical (PE idles until d0.. land):
            # progressive split so the first accum matmuls start ~7us in.
            splits = [2, 2, 4, 8, 16] if (b == 0 and c == 0) else [16, 16]
            d0 = 0
            for pi, step in enumerate(splits):
                nc.sync.dma_start(
                    out=xt_t[:, d0 * CH:(d0 + step) * CH].rearrange(
                        "p (n t) -> p n t", n=step),
                    in_=xt[128 * d0:128 * (d0 + step),
                           b * S + p0: b * S + p0 + CH].rearrange(
                        "(n p) t -> p n t", p=128))
                d0 += step
                for cb in (after_piece or {}).get(pi, ()):
                    cb()
            return xt_t

        def proj_unit(bt, b, c, kind, xt_t, alt=False):
            """One projection group: 32 accum matmuls + rope (or v-transpose).
            kind: 0..3 = q head j, 4 = k, 5 = v. Yields 5 steps.
            alt: prologue groups run back-to-back with no filler, so they
            alternate onto the (then idle) "out" bank instead of
            serializing on the single "qk" bank."""
            p0 = c * CH
            cosc = cos_sb[:, p0:p0 + CH]
            sinc = sin_sb[:, p0:p0 + CH]
            tag = "out" if (alt and kind % 2 == 1) else "qk"
            acc_ps = ps.tile([128, CH], F32, name="proj_ps", tag=tag,
                             bufs=2 if tag == "out" else 1)
            if kind < NQ:
                w_sb, wof = wqt_sb, kind * HD
                wstride = NQ * HD
            elif kind == NQ:
                w_sb, wof, wstride = wkt_sb, 0, HD
            else:
                w_sb, wof, wstride = wvt_sb, 0, HD
            for d0 in range(0, DT, 8):
                for d in range(d0, d0 + 8):
                    nc.tensor.matmul(
                        acc_ps,
                        lhsT=w_sb[:, d * wstride + wof:
                                  d * wstride + wof + HD],
                        rhs=xt_t[:, d * CH:(d + 1) * CH],
                        start=(d == 0), stop=(d == DT - 1))
                yield  # ~8 x 213ns PE quantum
            if kind < NQ:
                rope(bt["qb"], kind * S + p0, acc_ps, cosc, sinc, CH)
            elif kind == NQ:
                rope(bt["kb"], p0, acc_ps, cosc, sinc, CH)
            else:
                vcp = wk.tile([128, CH], BF16, name="vcp", tag="vcp", bufs=1)
                nc.scalar.copy(vcp, acc_ps)
                for tsub in range(CH // 128):
                    vtr_ps = ps.tile([128, 128], BF16, name="vtr_ps",
                                     tag="sc")
                    nc.tensor.transpose(vtr_ps,
                                        vcp[:, tsub * 128:(tsub + 1) * 128],
                                        ident)
                    col = (c * (CH // 128) + tsub) * 128
                    nc.vector.tensor_copy(bt["vb"][:, col:col + 128], vtr_ps)
            yield

        def attn_unit(bt, b, sc, h, epi_out):
            """Attention for (query-chunk sc, head h): 16 kt steps.
            Scores land [key, query] in PSUM; exp on ACT; denominator
            accumulates on DVE in fp16; all-ones matmul broadcasts sums.

            Pipelining (v3, NTFF-driven):
              - av(kt) is emitted one step behind scores(kt+1), so the PE
                never waits on the ACT exp stream mid-unit.
              - The epilogue (sums broadcast + reciprocal + normalize) is
                handed back via epi_out and emitted ~2 kt-steps into the
                NEXT unit, so its ACT copy does not delay exp(kt0') and the
                sums matmuls get runway before reading the DVE accs."""
            s0 = sc * SCW
            out_ps = ps.tile([128, SCW], F32, name="out_ps", tag="out")
            acc0 = wk.tile([128, SCW], FP16, name="acc0", tag="acc0", bufs=1)
            acc1 = wk.tile([128, SCW], FP16, name="acc1", tag="acc1", bufs=1)
            qsl = bt["qb"][:, h * S + s0:h * S + s0 + SCW]

            def kv(kt):
                if kt < PAST // 128:
                    return (bt["ckt"][:, kt * 128:(kt + 1) * 128],
                            bt["cv"][:, kt * 128:(kt + 1) * 128])
                kn = kt - PAST // 128
                return (bt["kb"][:, kn * 128:(kn + 1) * 128],
                        bt["vb"][:, kn * 128:(kn + 1) * 128])

            # kt-steps processed in PAIRS sharing one 2-bank PSUM tile so a
            # single [128,1024] exp covers both — halves ACT instruction
            # count/overhead (the exp stream otherwise paces attention).
            prev = None
            for p in range(NKT // 2):
                k0, v0 = kv(2 * p)
                k1, v1 = kv(2 * p + 1)
                sc_ps = ps.tile([128, 2 * SCW], F32, name="sc_ps", tag="sc")
                nc.tensor.matmul(sc_ps[:, 0:SCW], lhsT=k0, rhs=qsl)
                nc.tensor.matmul(sc_ps[:, SCW:2 * SCW], lhsT=k1, rhs=qsl)
                if prev is not None:
                    pexp, pv0, pv1 = prev
                    nc.tensor.matmul(out_ps, lhsT=pv0, rhs=pexp[:, 0:SCW],
                                     start=(p == 1), stop=False)
                    nc.tensor.matmul(out_ps, lhsT=pv1,
                                     rhs=pexp[:, SCW:2 * SCW],
                                     start=False, stop=False)
                exp_t = wk.tile([128, 2 * SCW], BF16, name="exp_t",
                                tag="exp", bufs=2)
                nc.scalar.activation(exp_t, sc_ps,
                                     mybir.ActivationFunctionType.Exp,
                                     scale=ISQRT_HD)
                if p == 0:
                    nc.vector.tensor_copy(acc0, exp_t[:, 0:SCW])
                    nc.vector.tensor_copy(acc1, exp_t[:, SCW:2 * SCW])
                else:
                    nc.vector.tensor_add(acc0, acc0, exp_t[:, 0:SCW])
                    nc.vector.tensor_add(acc1, acc1, exp_t[:, SCW:2 * SCW])
                prev = (exp_t, v0, v1)
                yield
            pexp, pv0, pv1 = prev
            nc.tensor.matmul(out_ps, lhsT=pv0, rhs=pexp[:, 0:SCW],
                             start=False, stop=False)
            nc.tensor.matmul(out_ps, lhsT=pv1, rhs=pexp[:, SCW:2 * SCW],
                             start=False, stop=True)

            def epilogue():
                sums_ps = ps.tile([128, SCW], F32, name="sums_ps", tag="sc")
                nc.tensor.matmul(sums_ps, lhsT=ones128, rhs=acc0,
                                 start=True, stop=False)
                nc.tensor.matmul(sums_ps, lhsT=ones128, rhs=acc1,
                                 start=False, stop=True)
                # Fast ACT copy frees the "sc" PSUM slot quickly; the
                # reciprocal (3.4us on real DVE) runs in-place afterwards,
                # off the PE-critical path — quartered so the first attnb
                # 128-token tile lands ~3us sooner (wo rows consume attnb
                # per 128-token tile and otherwise stall on batch 3's tail).
                inv_bc = wk.tile([128, SCW], BF16, name="inv_bc", tag="inv",
                                 bufs=1)
                nc.scalar.copy(inv_bc, sums_ps)
                for qtr in range(SCW // 128):
                    sl = slice(qtr * 128, (qtr + 1) * 128)
                    with nc.allow_low_precision(reason="softmax denom recip"):
                        nc.vector.reciprocal(inv_bc[:, sl], inv_bc[:, sl])
                    nc.vector.tensor_mul(
                        bt["attnb"][:, h * S + s0 + qtr * 128:
                                    h * S + s0 + (qtr + 1) * 128],
                        out_ps[:, sl], inv_bc[:, sl])
            epi_out.append(epilogue)
            yield

        NDC = DIM // SCW  # 8 wo column-groups per token tile
        HROW = NDC // 4   # groups per staging tile (quarter row)

        def wo_unit(bt, b, tt, dc, st_half, alt):
            """Partial x@wo.T for token-tile tt, 512 output dims dc.
            alt: when the proj fillers can no longer touch "qk" (all of the
            last batch, each batch's final chunk), odd groups borrow that
            idle bank so back-to-back wo groups don't serialize on the
            single "wo" bank waiting for the staging copy."""
            tag = "qk" if (alt and dc % 2 == 1) else "wo"
            wo_ps = ps.tile([128, SCW], F32, name="wo_ps", tag=tag,
                            bufs=1)
            for j in range(NQ):
                nc.tensor.matmul(
                    wo_ps,
                    lhsT=bt["attnb"][:, j * S + tt * 128:
                                     j * S + (tt + 1) * 128],
                    rhs=wot_sb[:, j * DIM + dc * SCW:
                               j * DIM + (dc + 1) * SCW],
                    start=(j == 0), stop=(j == NQ - 1))
            # GPSIMD cannot read PSUM.  Staging copies go to ACT (with
            # paired exps it is ~45% loaded; the DVE queue carries the
            # epilogue reciprocal bursts that would delay the copy and
            # stall the single-banked wo_ps slot).  In alt/drain phases the
            # qk-tagged groups copy on DVE so the two slots' frees ride
            # different queues.
            dst = st_half[:, (dc % HROW) * SCW:(dc % HROW + 1) * SCW]
            if alt and dc % 2 == 1:
                nc.vector.tensor_copy(dst, wo_ps)
            else:
                nc.scalar.copy(dst, wo_ps)
            if dc % HROW == HROW - 1:
                row = (b * (S // 128) + tt) * 128
                col = (dc // HROW) * HROW * SCW
                nc.sync.dma_start(
                    out=out_p[row:row + 128, col:col + HROW * SCW],
                    in_=st_half)
            yield

        def wo_row(bt, b, tt, alt):
            for part in range(NDC // HROW):
                st_half = wk.tile([128, HROW * SCW], BF16, name="st_half",
                                  tag="st", bufs=2)
                for dc in range(part * HROW, (part + 1) * HROW):
                    yield from wo_unit(bt, b, tt, dc, st_half, alt)

        # ================= scheduler ======================================
        # Per batch: proj(b) is fully emitted before attn(b).  attn(b)'s 8
        # units are ACT-paced; between kt-steps we interleave filler steps
        # from wo(b) rows (ready once all heads of their s-chunk are done)
        # and proj(b+1) (+ its xt prefetch).  All wo(b) rows are emitted
        # before attn(b+1) starts so attnb can be single-buffered.
        bts = [None] * B
        xts = [None] * B

        def cache_load(b):
            bt = bts[b]
            nc.sync.dma_start(out=bt["ckt"], in_=ckt[b, :, :])
            nc.sync.dma_start(
                out=bt["cv"].rearrange("p (n d) -> p n d", n=PAST // 128),
                in_=cv[b, :, :].rearrange("(n p) t -> p n t", p=128))

        def proj_batch(b):
            """Generator over proj(b) steps (incl. xt loads + cache DMA).
            The kv-cache DMA is issued after the first xt chunk so it does
            not delay the projection-critical loads on the in-order queue.
            Heads q2/q3 are deferred (see projq23_batch): they are only
            needed by attention units h2/h3, so they serve as attn-phase
            filler — which matters for the last batch, whose attention has
            no next-batch projections to interleave."""
            bts[b] = batch_tiles(b)
            xts[b] = [None] * NCH_B
            for c in range(NCH_B):
                xts[b][c] = xt_load(b, c)
                if b == 0:
                    if c == 0:
                        load_wvt()
                        load_cos_sin()
                        load_wqt()
                    else:
                        load_wot()
                        cache_load(b)
                elif c == 0:
                    cache_load(b)
                for kind in (NQ, NQ + 1, 0, 1):  # k, v first
                    yield from proj_unit(bts[b], b, c, kind, xts[b][c],
                                         alt=(b == 0))

        def projq23_batch(b):
            for c in range(NCH_B):
                for kind in (2, 3):
                    yield from proj_unit(bts[b], b, c, kind, xts[b][c])

        def attn_batch(b):
            epis = []        # deferred epilogue of the previous unit
            pending_sc = None  # sc-chunk whose last epilogue is deferred
            for sc in range(S // SCW):
                for h in range(NQ if do_attn else 0):
                    step_n = 0
                    for _ in attn_unit(bts[b], b, sc, h, epis):
                        yield
                        step_n += 1
                        if step_n == 2 and epis:
                            epis.pop(0)()
                            if pending_sc is not None:
                                yield ("sc_done", pending_sc)
                                pending_sc = None
                pending_sc = sc
            while epis:
                epis.pop(0)()
            if pending_sc is not None:
                yield ("sc_done", pending_sc)

        def wo_rows(b, tts, alt=False):
            for tt in tts:
                yield from wo_row(bts[b], b, tt, alt)

        # prologue: batch 0 projections (k/v/q0/q1), no filler available
        for _ in proj_batch(0):
            pass

        for b in range(B):
            # Filler queues: proj-ish (deferred q2/q3 of batch b, then
            # proj(b+1)) and wo rows of batch b (gated on sc completion).
            # Strict FIFO within proj-ish (q23 before proj(b+1)): proj(b+1)'s
            # xt DMA reuses q23(b)'s xt slot, so emitting it before all
            # q23(b) matmuls are on the PE queue would deadlock the in-order
            # queues.  Between classes we ALTERNATE: consecutive proj groups
            # otherwise stall ~3us each on the "qk" PSUM slot, whose freeing
            # ACT copy sits behind the exp stream (measured via NTFF).
            pf = [projq23_batch(b)]
            if b + 1 < B:
                pf.append(proj_batch(b + 1))
            wf = []
            rr = [0]

            def drain_filler(n, pf=pf, wf=wf, rr=rr):
                done = 0
                while done < n and (pf or wf):
                    use_wf = bool(wf) and (not pf or rr[0] % 2 == 1)
                    rr[0] += 1
                    gen = wf[0] if use_wf else pf[0]
                    try:
                        next(gen)
                        done += 1
                    except StopIteration:
                        (wf if use_wf else pf).remove(gen)
                return done

            if do_attn:
                for step in attn_batch(b):
                    if isinstance(step, tuple) and step[0] == "sc_done":
                        sc = step[1]
                        if do_wo:
                            tts = range(sc * (SCW // 128),
                                        (sc + 1) * (SCW // 128))
                            alt = (b == B - 1) or (sc == S // SCW - 1)
                            wf.append(wo_rows(b, tts, alt))
                        continue
                    # one attn pair-step emitted (4 matmuls); ~2 filler
                    # steps.  The last batch has only ~84 filler steps for
                    # 72 yields — drain 1 so they stretch across the batch.
                    drain_filler(2 if b + 1 < B else 1)
            elif do_wo:
                wf.append(wo_rows(b, range(S // 128), alt=True))
            # drain all remaining wo(b) (+ any proj(b+1) leftovers)
            while drain_filler(64):
                pass


def _rope_perm():
    # even features first, then odd — per 128-wide head
    return np.concatenate([np.arange(0, HD, 2), np.arange(1, HD, 2)])


def _prep_inputs(x, freqs_cos, freqs_sin, cache_k, cache_v, wq, wk, wv, wo):
    perm = _rope_perm()
    xt = np.ascontiguousarray(
        x.reshape(T, DIM).T).astype(NP_BF16)
    cos_t = np.ascontiguousarray(freqs_cos.T).astype(NP_BF16)
    sin_t = np.ascontiguousarray(freqs_sin.T).astype(NP_BF16)

    in_maps = []
    for m in range(NCORES):
        wq_m = wq[m * NQ * HD:(m + 1) * NQ * HD]  # (512, 4096)
        wq_m = wq_m.reshape(NQ, HD, DIM)[:, perm, :].reshape(NQ * HD, DIM)
        wqt_m = np.ascontiguousarray(wq_m.T).astype(NP_BF16)
        wk_m = wk[m * HD:(m + 1) * HD][perm]
        wkt_m = np.ascontiguousarray(wk_m.T).astype(NP_BF16)
        wv_m = wv[m * HD:(m + 1) * HD]
        wvt_m = np.ascontiguousarray(wv_m.T).astype(NP_BF16)
        wot_m = np.ascontiguousarray(
            wo[:, m * NQ * HD:(m + 1) * NQ * HD].T).astype(NP_BF16)
        ckt_m = np.ascontiguousarray(
            cache_k[:, m][:, :, perm].transpose(0, 2, 1)).astype(NP_BF16)
        cv_m = np.ascontiguousarray(cache_v[:, m]).astype(NP_BF16)
        in_maps.append({
            "xt": xt, "wqt": wqt_m, "wkt": wkt_m, "wvt": wvt_m,
            "wot": wot_m, "ckt": ckt_m, "cv": cv_m,
            "cos": cos_t, "sin": sin_t,
        })
    return in_maps


def kernel(x, freqs_cos, freqs_sin, cache_k, cache_v, wq, wk, wv, wo):
    global LAST_EXEC_NS, LAST_RESULTS
    if "nc" not in _CACHED:
        _CACHED["nc"] = _build_nc()
    nc = _CACHED["nc"]

    in_maps = _prep_inputs(np.asarray(x), np.asarray(freqs_cos),
                           np.asarray(freqs_sin), np.asarray(cache_k),
                           np.asarray(cache_v), np.asarray(wq),
                           np.asarray(wk), np.asarray(wv), np.asarray(wo))

    trace = os.environ.get("KERNEL_TRACE", "0") == "1"
    try:
        res = run_bass_kernel_spmd(nc, in_maps, core_ids=list(range(NCORES)),
                                   trace=trace)
    except (ImportError, ModuleNotFoundError):
        # NTFF profiling hook unavailable in this environment
        res = run_bass_kernel_spmd(nc, in_maps, core_ids=list(range(NCORES)),
                                   trace=False)
    LAST_EXEC_NS = res.exec_time_ns
    LAST_RESULTS = res

    total = np.zeros((T, DIM), dtype=np.float32)
    for r in res.results:
        total += r["out_p"].astype(np.float32)
    return total.reshape(B, S, DIM)



# revision 45
# speedup vs baseline: 1.0123x; 1.0029x over previous
"""Multi-head attention (GQA prefill with KV cache) on 8 trn2 NeuronCores.

Sharding: tensor-parallel over heads. Core m owns KV head m (of 8) and the
4 query heads 4m..4m+3.  Each core computes its heads' attention output and
a partial x @ wo.T contribution; the host sums the 8 partials.

Layout notes:
  - All activations on device are "feature-major" ([feature, token]) so the
    token dim rides the matmul moving dim; x is transposed on the host once.
  - RoPE's (even, odd) pair interleave is removed by permuting wq/wk rows and
    cache_k's head_dim on the host (QK^T is invariant to a shared permutation
    of head_dim), so on device RoPE is plain 64-partition block arithmetic.
  - Scores are computed transposed ([key, query]) so the softmax denominator
    is an all-ones matmul and the attention output lands feature-major,
    which feeds the wo matmul directly.

Pipeline notes (v2):
  - PSUM: four independent 1-bank tags (qk / sc / out / wo), bufs=2 each,
    so the three phases never serialize on pool rotation.
  - Softmax denominators accumulate on the (otherwise idle) DVE in fp16;
    a single all-ones [128,128] matmul broadcasts the per-query sums to all
    partitions, replacing 16 ones-vector matmuls per attention unit.
  - Emission is software-pipelined at sub-unit granularity: projection and
    wo-projection matmul groups are interleaved between attention kt-steps,
    keeping PE busy while the exp stream (ACT-paced) advances.
  - Output partials are written bf16, one DMA per 128-token row.
"""

import os
import sys

import numpy as np

if "/opt/trn_rl_repo" not in sys.path:
    sys.path.insert(0, "/opt/trn_rl_repo")

import ml_dtypes

import concourse.bass as bass
import concourse.mybir as mybir
import concourse.tile as tile
from concourse.bass_utils import run_bass_kernel_spmd
from concourse.masks import make_identity

BF16 = mybir.dt.bfloat16
FP16 = mybir.dt.float16
F32 = mybir.dt.float32
NP_BF16 = ml_dtypes.bfloat16

B, S, DIM = 4, 1024, 4096
N_HEADS, N_KV_HEADS = 32, 8
HD = 128
PAST = 1024
NCORES = 8
NQ = N_HEADS // NCORES  # 4 q heads per core
T = B * S  # 4096 tokens
DT = DIM // 128  # 32 contraction tiles
CH = 512  # projection token-chunk
NCH_B = S // CH  # chunks per batch (2)
SCW = 512  # attention s-chunk width
NKT = (PAST + S) // 128  # 16 key tiles per batch
ISQRT_HD = 1.0 / float(np.sqrt(HD))

LAST_EXEC_NS = None
LAST_RESULTS = None

_CACHED = {}


def _split_multi_waits(nc):
    """walrus' per-instruction sync encoding fits one wait; hoist extras
    onto standalone EventSemaphore instructions on the same engine queue."""
    for f in nc.m.functions:
        for blk in f.blocks:
            insts = blk.instructions
            if not any(i.sync_info and i.sync_info.on_wait
                       and len(i.sync_info.on_wait) > 1 for i in insts):
                continue
            new = []
            for inst in insts:
                si = inst.sync_info
                if si is not None and si.on_wait and len(si.on_wait) > 1:
                    waits = list(si.on_wait)
                    for wt in waits[:-1]:
                        evs = mybir.InstEventSemaphore(
                            name=f"I-wsplit-{nc.next_id()}", ins=[], outs=[])
                        evs.engine = inst.engine
                        evs.sync_info = mybir.SyncInfo(on_wait=[wt],
                                                       on_update=[])
                        new.append(evs)
                    inst.sync_info = mybir.SyncInfo(
                        on_wait=[waits[-1]],
                        on_update=list(si.on_update or []))
                new.append(inst)
            insts[:] = new


def _build_nc(split_waits=True, mode="full", reps=1):
    nc = bass.Bass("TRN2", target_bir_lowering=False, debug=False,
                   num_devices=NCORES)

    xt = nc.dram_tensor("xt", [DIM, T], BF16, kind="ExternalInput")
    wqt = nc.dram_tensor("wqt", [DIM, NQ * HD], BF16, kind="ExternalInput")
    wkt = nc.dram_tensor("wkt", [DIM, HD], BF16, kind="ExternalInput")
    wvt = nc.dram_tensor("wvt", [DIM, HD], BF16, kind="ExternalInput")
    wot = nc.dram_tensor("wot", [NQ * HD, DIM], BF16, kind="ExternalInput")
    ckt = nc.dram_tensor("ckt", [B, HD, PAST], BF16, kind="ExternalInput")
    cv = nc.dram_tensor("cv", [B, PAST, HD], BF16, kind="ExternalInput")
    cos = nc.dram_tensor("cos", [HD // 2, S], BF16, kind="ExternalInput")
    sin = nc.dram_tensor("sin", [HD // 2, S], BF16, kind="ExternalInput")
    out_p = nc.dram_tensor("out_p", [T, DIM], BF16, kind="ExternalOutput")

    with tile.TileContext(nc) as tc:
        if mode == "null":
            _emit_null(tc, nc)
        else:
            for _ in range(reps):
                _emit(tc, nc, xt, wqt, wkt, wvt, wot, ckt, cv, cos, sin,
                      out_p, mode=mode)
    if split_waits:
        _split_multi_waits(nc)
    return nc


def _emit_null(tc, nc):
    """Near-empty program: measures per-dispatch overhead in bench.py."""
    from contextlib import ExitStack
    with ExitStack() as ctx:
        cw = ctx.enter_context(tc.tile_pool(name="nullp", bufs=1))
        t = cw.tile([128, 128], BF16, name="null_t")
        nc.vector.memset(t, 0.0)


def _emit(tc, nc, xt, wqt, wkt, wvt, wot, ckt, cv, cos, sin, out_p,
          mode="full"):
    from contextlib import ExitStack
    do_attn = mode in ("full", "bc")
    do_wo = mode in ("full", "bd")

    with ExitStack() as ctx:
        cw = ctx.enter_context(tc.tile_pool(name="consts", bufs=1))
        pb = ctx.enter_context(tc.tile_pool(name="perbatch", bufs=2))
        wk = ctx.enter_context(tc.tile_pool(name="work", bufs=2))
        ps = ctx.enter_context(tc.tile_pool(name="ps", bufs=2, space="PSUM"))

        # ---- resident constants -------------------------------------------
        # DMA order is startup-critical: the first emitted work is the k/v
        # projection of batch 0 chunk 0, which needs only wkt/wvt (+ the xt
        # chunk, issued inside proj_batch).  wqt follows (needed ~15us later
        # by the first q unit); wot and the kv-cache much later.
        wkt_sb = cw.tile([128, DT * HD], BF16, name="wkt_sb")
        nc.sync.dma_start(
            out=wkt_sb.rearrange("p (n j) -> p n j", n=DT),
            in_=wkt[:, :].rearrange("(n p) j -> p n j", p=128))
        wvt_sb = cw.tile([128, DT * HD], BF16, name="wvt_sb")

        def load_wvt():
            nc.sync.dma_start(
                out=wvt_sb.rearrange("p (n j) -> p n j", n=DT),
                in_=wvt[:, :].rearrange("(n p) j -> p n j", p=128))
        # cos/sin duplicated across both 64-partition halves so RoPE's two
        # multiplies can run full-width: q*cos gives (r*cos | i*cos).
        # (loaded after the first xt chunk — see proj_batch)
        cos_sb = cw.tile([128, S], BF16, name="cos_sb")
        sin_sb = cw.tile([128, S], BF16, name="sin_sb")

        def load_cos_sin():
            nc.sync.dma_start(out=cos_sb[0:64, :], in_=cos[:, :])
            nc.sync.dma_start(out=cos_sb[64:128, :], in_=cos[:, :])
            nc.sync.dma_start(out=sin_sb[0:64, :], in_=sin[:, :])
            nc.sync.dma_start(out=sin_sb[64:128, :], in_=sin[:, :])

        ones128 = cw.tile([128, 128], FP16, name="ones128")
        nc.vector.memset(ones128, 1.0)
        ident = cw.tile([128, 128], BF16, name="ident")
        make_identity(nc, ident)
        wqt_sb = cw.tile([128, DT * NQ * HD], BF16, name="wqt_sb")
        wot_sb = cw.tile([128, NQ * DIM], BF16, name="wot_sb")

        def load_wqt():
            nc.sync.dma_start(
                out=wqt_sb.rearrange("p (n j) -> p n j", n=DT),
                in_=wqt[:, :].rearrange("(n p) j -> p n j", p=128))

        def load_wot():
            nc.sync.dma_start(
                out=wot_sb.rearrange("p (n d) -> p n d", n=NQ),
                in_=wot[:, :].rearrange("(n p) d -> p n d", p=128))

        # ---- per-batch / work tiles (slot handles, rotated via tags) ------
        def batch_tiles(b):
            qb_t = pb.tile([128, NQ * S], BF16, name="qb", tag="qb")
            kb_t = pb.tile([128, S], BF16, name="kb", tag="kb")
            vb_t = pb.tile([128, S], BF16, name="vb", tag="vb")
            attnb_t = pb.tile([128, NQ * S], BF16, name="attnb", tag="attnb",
                              bufs=1)
            ckt_b = pb.tile([128, PAST], BF16, name="ckt_b", tag="ckt_b")
            cv_b = pb.tile([128, PAST], BF16, name="cv_b", tag="cv_b")
            return dict(qb=qb_t, kb=kb_t, vb=vb_t, attnb=attnb_t,
                        ckt=ckt_b, cv=cv_b)

        def rope(dst_tile, dst_col, src_ps, cosc, sinc, n):
            """src layout (r|i) on partition halves.
            dst[0:64] = r*cos - i*sin ; dst[64:128] = r*sin + i*cos.
            A fast ACT copy stages src to fp16 SBUF first, freeing the "qk"
            PSUM slot immediately (the DVE rope ops otherwise queue behind
            attention acc-adds and stall the next proj group's matmuls).
            Two full-width muls: tc = (r*cos | i*cos), ts = (r*sin | i*sin),
            then dst_r = tc_hi - ts_lo, dst_i = ts_hi + tc_lo."""
            qraw = wk.tile([128, CH], FP16, name="qraw", tag="qraw", bufs=2)
            nc.scalar.copy(qraw[:, :n], src_ps)
            tc_ = wk.tile([128, CH], FP16, name="rope_tc", tag="rope_tc",
                          bufs=1)
            ts_ = wk.tile([128, CH], FP16, name="rope_ts", tag="rope_ts",
                          bufs=1)
            nc.vector.tensor_mul(tc_[:, :n], qraw[:, :n], cosc)
            nc.vector.tensor_mul(ts_[0:64, :n], qraw[64:128, :n],
                                 sinc[64:128, :])
            nc.vector.tensor_mul(ts_[64:128, :n], qraw[0:64, :n],
                                 sinc[0:64, :])
            nc.vector.tensor_sub(dst_tile[0:64, dst_col:dst_col + n],
                                 tc_[0:64, :n], ts_[0:64, :n])
            nc.vector.tensor_add(dst_tile[64:128, dst_col:dst_col + n],
                                 ts_[64:128, :n], tc_[64:128, :n])

        # ================= work-unit generators (each yields steps) ========
        def xt_load(b, c, after_piece=None):
            """DMA one 512-token chunk of x into SBUF (feature-major).
            Split so the projection's first accum matmuls can start as soon
            as the first piece lands (subtile deps).  after_piece maps piece
            index -> load callbacks interleaved onto the (serial) DMA ring
            by the time their consumer actually needs them."""
            xt_t = wk.tile([128, DT * CH], BF16, name="xt_t", tag="xt")
            p0 = c * CH
            # First chunk is latency-critical (PE idles until d0.. land):
            # progressive split so the first accum matmuls start ~7us in.
            splits = [2, 2, 4, 8, 16] if (b == 0 and c == 0) else [16, 16]
            d0 = 0
            for pi, step in enumerate(splits):
                nc.sync.dma_start(
                    out=xt_t[:, d0 * CH:(d0 + step) * CH].rearrange(
                        "p (n t) -> p n t", n=step),
                    in_=xt[128 * d0:128 * (d0 + step),
                           b * S + p0: b * S + p0 + CH].rearrange(
                        "(n p) t -> p n t", p=128))
                d0 += step
                for cb in (after_piece or {}).get(pi, ()):
                    cb()
            return xt_t

        def proj_unit(bt, b, c, kind, xt_t, alt=False):
            """One projection group: 32 accum matmuls + rope (or v-transpose).
            kind: 0..3 = q head j, 4 = k, 5 = v. Yields 5 steps.
            alt: prologue groups run back-to-back with no filler, so they
            alternate onto the (then idle) "out" bank instead of
            serializing on the single "qk" bank."""
            p0 = c * CH
            cosc = cos_sb[:, p0:p0 + CH]
            sinc = sin_sb[:, p0:p0 + CH]
            tag = "out" if (alt and kind % 2 == 1) else "qk"
            acc_ps = ps.tile([128, CH], F32, name="proj_ps", tag=tag,
                             bufs=2 if tag == "out" else 1)
            if kind < NQ:
                w_sb, wof = wqt_sb, kind * HD
                wstride = NQ * HD
            elif kind == NQ:
                w_sb, wof, wstride = wkt_sb, 0, HD
            else:
                w_sb, wof, wstride = wvt_sb, 0, HD
            for d0 in range(0, DT, 8):
                for d in range(d0, d0 + 8):
                    nc.tensor.matmul(
                        acc_ps,
                        lhsT=w_sb[:, d * wstride + wof:
                                  d * wstride + wof + HD],
                        rhs=xt_t[:, d * CH:(d + 1) * CH],
                        start=(d == 0), stop=(d == DT - 1))
                yield  # ~8 x 213ns PE quantum
            if kind < NQ:
                rope(bt["qb"], kind * S + p0, acc_ps, cosc, sinc, CH)
            elif kind == NQ:
                rope(bt["kb"], p0, acc_ps, cosc, sinc, CH)
            else:
                vcp = wk.tile([128, CH], BF16, name="vcp", tag="vcp", bufs=1)
                nc.scalar.copy(vcp, acc_ps)
                for tsub in range(CH // 128):
                    vtr_ps = ps.tile([128, 128], BF16, name="vtr_ps",
                                     tag="sc")
                    nc.tensor.transpose(vtr_ps,
                                        vcp[:, tsub * 128:(tsub + 1) * 128],
                                        ident)
                    col = (c * (CH // 128) + tsub) * 128
                    nc.vector.tensor_copy(bt["vb"][:, col:col + 128], vtr_ps)
            yield

        def attn_unit(bt, b, sc, h, epi_out):
            """Attention for (query-chunk sc, head h): 16 kt steps.
            Scores land [key, query] in PSUM; exp on ACT; denominator
            accumulates on DVE in fp16; all-ones matmul broadcasts sums.

            Pipelining (v3, NTFF-driven):
              - av(kt) is emitted one step behind scores(kt+1), so the PE
                never waits on the ACT exp stream mid-unit.
              - The epilogue (sums broadcast + reciprocal + normalize) is
                handed back via epi_out and emitted ~2 kt-steps into the
                NEXT unit, so its ACT copy does not delay exp(kt0') and the
                sums matmuls get runway before reading the DVE accs."""
            s0 = sc * SCW
            out_ps = ps.tile([128, SCW], F32, name="out_ps", tag="out")
            acc0 = wk.tile([128, SCW], FP16, name="acc0", tag="acc0", bufs=1)
            acc1 = wk.tile([128, SCW], FP16, name="acc1", tag="acc1", bufs=1)
            qsl = bt["qb"][:, h * S + s0:h * S + s0 + SCW]

            def kv(kt):
                if kt < PAST // 128:
                    return (bt["ckt"][:, kt * 128:(kt + 1) * 128],
                            bt["cv"][:, kt * 128:(kt + 1) * 128])
                kn = kt - PAST // 128
                return (bt["kb"][:, kn * 128:(kn + 1) * 128],
                        bt["vb"][:, kn * 128:(kn + 1) * 128])

            # kt-steps processed in PAIRS sharing one 2-bank PSUM tile so a
            # single [128,1024] exp covers both — halves ACT instruction
            # count/overhead (the exp stream otherwise paces attention).
            prev = None
            for p in range(NKT // 2):
                k0, v0 = kv(2 * p)
                k1, v1 = kv(2 * p + 1)
                sc_ps = ps.tile([128, 2 * SCW], F32, name="sc_ps", tag="sc")
                nc.tensor.matmul(sc_ps[:, 0:SCW], lhsT=k0, rhs=qsl)
                nc.tensor.matmul(sc_ps[:, SCW:2 * SCW], lhsT=k1, rhs=qsl)
                if prev is not None:
                    pexp, pv0, pv1 = prev
                    nc.tensor.matmul(out_ps, lhsT=pv0, rhs=pexp[:, 0:SCW],
                                     start=(p == 1), stop=False)
                    nc.tensor.matmul(out_ps, lhsT=pv1,
                                     rhs=pexp[:, SCW:2 * SCW],
                                     start=False, stop=False)
                exp_t = wk.tile([128, 2 * SCW], BF16, name="exp_t",
                                tag="exp", bufs=2)
                nc.scalar.activation(exp_t, sc_ps,
                                     mybir.ActivationFunctionType.Exp,
                                     scale=ISQRT_HD)
                if p == 0:
                    nc.vector.tensor_copy(acc0, exp_t[:, 0:SCW])
                    nc.vector.tensor_copy(acc1, exp_t[:, SCW:2 * SCW])
                else:
                    nc.vector.tensor_add(acc0, acc0, exp_t[:, 0:SCW])
                    nc.vector.tensor_add(acc1, acc1, exp_t[:, SCW:2 * SCW])
                prev = (exp_t, v0, v1)
                yield
            pexp, pv0, pv1 = prev
            nc.tensor.matmul(out_ps, lhsT=pv0, rhs=pexp[:, 0:SCW],
                             start=False, stop=False)
            nc.tensor.matmul(out_ps, lhsT=pv1, rhs=pexp[:, SCW:2 * SCW],
                             start=False, stop=True)

            def epilogue():
                sums_ps = ps.tile([128, SCW], F32, name="sums_ps", tag="sc")
                nc.tensor.matmul(sums_ps, lhsT=ones128, rhs=acc0,
                                 start=True, stop=False)
                nc.tensor.matmul(sums_ps, lhsT=ones128, rhs=acc1,
                                 start=False, stop=True)
                # Fast ACT copy frees the "sc" PSUM slot quickly; the
                # reciprocal (3.4us on real DVE) runs in-place afterwards,
                # off the PE-critical path — quartered so the first attnb
                # 128-token tile lands ~3us sooner (wo rows consume attnb
                # per 128-token tile and otherwise stall on batch 3's tail).
                inv_bc = wk.tile([128, SCW], BF16, name="inv_bc", tag="inv",
                                 bufs=1)
                nc.scalar.copy(inv_bc, sums_ps)
                for qtr in range(SCW // 128):
                    sl = slice(qtr * 128, (qtr + 1) * 128)
                    with nc.allow_low_precision(reason="softmax denom recip"):
                        nc.vector.reciprocal(inv_bc[:, sl], inv_bc[:, sl])
                    nc.vector.tensor_mul(
                        bt["attnb"][:, h * S + s0 + qtr * 128:
                                    h * S + s0 + (qtr + 1) * 128],
                        out_ps[:, sl], inv_bc[:, sl])
            epi_out.append(epilogue)
            yield

        NDC = DIM // SCW  # 8 wo column-groups per token tile
        HROW = NDC // 4   # groups per staging tile (quarter row)

        def wo_unit(bt, b, tt, dc, st_half, alt):
            """Partial x@wo.T for token-tile tt, 512 output dims dc.
            alt: when the proj fillers can no longer touch "qk" (all of the
            last batch, each batch's final chunk), odd groups borrow that
            idle bank so back-to-back wo groups don't serialize on the
            single "wo" bank waiting for the staging copy."""
            tag = "qk" if (alt and dc % 2 == 1) else "wo"
            wo_ps = ps.tile([128, SCW], F32, name="wo_ps", tag=tag,
                            bufs=1)
            for j in range(NQ):
                nc.tensor.matmul(
                    wo_ps,
                    lhsT=bt["attnb"][:, j * S + tt * 128:
                                     j * S + (tt + 1) * 128],
                    rhs=wot_sb[:, j * DIM + dc * SCW:
                               j * DIM + (dc + 1) * SCW],
                    start=(j == 0), stop=(j == NQ - 1))
            # GPSIMD cannot read PSUM.  All staging copies go to ACT: with
            # paired exps ACT is ~45% loaded, while the DVE queue carries
            # the epilogue reciprocal bursts that would delay the copy and
            # stall the single-banked wo_ps slot (measured via NTFF).
            dst = st_half[:, (dc % HROW) * SCW:(dc % HROW + 1) * SCW]
            nc.scalar.copy(dst, wo_ps)
            if dc % HROW == HROW - 1:
                row = (b * (S // 128) + tt) * 128
                col = (dc // HROW) * HROW * SCW
                nc.sync.dma_start(
                    out=out_p[row:row + 128, col:col + HROW * SCW],
                    in_=st_half)
            yield

        def wo_row(bt, b, tt, alt):
            for part in range(NDC // HROW):
                st_half = wk.tile([128, HROW * SCW], BF16, name="st_half",
                                  tag="st", bufs=2)
                for dc in range(part * HROW, (part + 1) * HROW):
                    yield from wo_unit(bt, b, tt, dc, st_half, alt)

        # ================= scheduler ======================================
        # Per batch: proj(b) is fully emitted before attn(b).  attn(b)'s 8
        # units are ACT-paced; between kt-steps we interleave filler steps
        # from wo(b) rows (ready once all heads of their s-chunk are done)
        # and proj(b+1) (+ its xt prefetch).  All wo(b) rows are emitted
        # before attn(b+1) starts so attnb can be single-buffered.
        bts = [None] * B
        xts = [None] * B

        def cache_load(b):
            bt = bts[b]
            nc.sync.dma_start(out=bt["ckt"], in_=ckt[b, :, :])
            nc.sync.dma_start(
                out=bt["cv"].rearrange("p (n d) -> p n d", n=PAST // 128),
                in_=cv[b, :, :].rearrange("(n p) t -> p n t", p=128))

        def proj_batch(b):
            """Generator over proj(b) steps (incl. xt loads + cache DMA).
            The kv-cache DMA is issued after the first xt chunk so it does
            not delay the projection-critical loads on the in-order queue.
            Heads q2/q3 are deferred (see projq23_batch): they are only
            needed by attention units h2/h3, so they serve as attn-phase
            filler — which matters for the last batch, whose attention has
            no next-batch projections to interleave."""
            bts[b] = batch_tiles(b)
            xts[b] = [None] * NCH_B
            for c in range(NCH_B):
                xts[b][c] = xt_load(b, c)
                if b == 0:
                    if c == 0:
                        load_wvt()
                        load_cos_sin()
                        load_wqt()
                    else:
                        load_wot()
                        cache_load(b)
                elif c == 0:
                    cache_load(b)
                for kind in (NQ, NQ + 1, 0, 1):  # k, v first
                    yield from proj_unit(bts[b], b, c, kind, xts[b][c],
                                         alt=(b == 0))

        def projq23_batch(b):
            for c in range(NCH_B):
                for kind in (2, 3):
                    yield from proj_unit(bts[b], b, c, kind, xts[b][c])

        def attn_batch(b):
            epis = []        # deferred epilogue of the previous unit
            pending_sc = None  # sc-chunk whose last epilogue is deferred
            for sc in range(S // SCW):
                for h in range(NQ if do_attn else 0):
                    step_n = 0
                    for _ in attn_unit(bts[b], b, sc, h, epis):
                        yield
                        step_n += 1
                        if step_n == 2 and epis:
                            epis.pop(0)()
                            if pending_sc is not None:
                                yield ("sc_done", pending_sc)
                                pending_sc = None
                pending_sc = sc
            while epis:
                epis.pop(0)()
            if pending_sc is not None:
                yield ("sc_done", pending_sc)

        def wo_rows(b, tts, alt=False):
            for tt in tts:
                yield from wo_row(bts[b], b, tt, alt)

        # prologue: batch 0 projections (k/v/q0/q1), no filler available
        for _ in proj_batch(0):
            pass

        for b in range(B):
            # Filler queues: proj-ish (deferred q2/q3 of batch b, then
            # proj(b+1)) and wo rows of batch b (gated on sc completion).
            # Strict FIFO within proj-ish (q23 before proj(b+1)): proj(b+1)'s
            # xt DMA reuses q23(b)'s xt slot, so emitting it before all
            # q23(b) matmuls are on the PE queue would deadlock the in-order
            # queues.  Between classes we ALTERNATE: consecutive proj groups
            # otherwise stall ~3us each on the "qk" PSUM slot, whose freeing
            # ACT copy sits behind the exp stream (measured via NTFF).
            pf = [projq23_batch(b)]
            if b + 1 < B:
                pf.append(proj_batch(b + 1))
            wf = []
            rr = [0]

            def drain_filler(n, pf=pf, wf=wf, rr=rr):
                done = 0
                while done < n and (pf or wf):
                    use_wf = bool(wf) and (not pf or rr[0] % 2 == 1)
                    rr[0] += 1
                    gen = wf[0] if use_wf else pf[0]
                    try:
                        next(gen)
                        done += 1
                    except StopIteration:
                        (wf if use_wf else pf).remove(gen)
                return done

            if do_attn:
                for step in attn_batch(b):
                    if isinstance(step, tuple) and step[0] == "sc_done":
                        sc = step[1]
                        if do_wo:
                            tts = range(sc * (SCW // 128),
                                        (sc + 1) * (SCW // 128))
                            alt = (b == B - 1) or (sc == S // SCW - 1)
                            wf.append(wo_rows(b, tts, alt))
                        continue
                    # one attn pair-step emitted (4 matmuls); ~2 filler
                    # steps.  The last batch has only ~84 filler steps for
                    # 72 yields — drain 1 so they stretch across the batch.
                    drain_filler(2 if b + 1 < B else 1)
            elif do_wo:
                wf.append(wo_rows(b, range(S // 128), alt=True))
            # drain all remaining wo(b) (+ any proj(b+1) leftovers)
            while drain_filler(64):
                pass


def _rope_perm():
    # even features first, then odd — per 128-wide head
    return np.concatenate([np.arange(0, HD, 2), np.arange(1, HD, 2)])


def _prep_inputs(x, freqs_cos, freqs_sin, cache_k, cache_v, wq, wk, wv, wo):
    perm = _rope_perm()
    xt = np.ascontiguousarray(
        x.reshape(T, DIM).T).astype(NP_BF16)
    cos_t = np.ascontiguousarray(freqs_cos.T).astype(NP_BF16)
    sin_t = np.ascontiguousarray(freqs_sin.T).astype(NP_BF16)

    in_maps = []
    for m in range(NCORES):
        wq_m = wq[m * NQ * HD:(m + 1) * NQ * HD]  # (512, 4096)
        wq_m = wq_m.reshape(NQ, HD, DIM)[:, perm, :].reshape(NQ * HD, DIM)
        wqt_m = np.ascontiguousarray(wq_m.T).astype(NP_BF16)
        wk_m = wk[m * HD:(m + 1) * HD][perm]
        wkt_m = np.ascontiguousarray(wk_m.T).astype(NP_BF16)
        wv_m = wv[m * HD:(m + 1) * HD]
        wvt_m = np.ascontiguousarray(wv_m.T).astype(NP_BF16)
        wot_m = np.ascontiguousarray(
            wo[:, m * NQ * HD:(m + 1) * NQ * HD].T).astype(NP_BF16)
        ckt_m = np.ascontiguousarray(
            cache_k[:, m][:, :, perm].transpose(0, 2, 1)).astype(NP_BF16)
        cv_m = np.ascontiguousarray(cache_v[:, m]).astype(NP_BF16)
        in_maps.append({
            "xt": xt, "wqt": wqt_m, "wkt": wkt_m, "wvt": wvt_m,
            "wot": wot_m, "ckt": ckt_m, "cv": cv_m,
            "cos": cos_t, "sin": sin_t,
        })
    return in_maps


def kernel(x, freqs_cos, freqs_sin, cache_k, cache_v, wq, wk, wv, wo):
    global LAST_EXEC_NS, LAST_RESULTS
    if "nc" not in _CACHED:
        _CACHED["nc"] = _build_nc()
    nc = _CACHED["nc"]

    in_maps = _prep_inputs(np.asarray(x), np.asarray(freqs_cos),
                           np.asarray(freqs_sin), np.asarray(cache_k),
                           np.asarray(cache_v), np.asarray(wq),
                           np.asarray(wk), np.asarray(wv), np.asarray(wo))

    trace = os.environ.get("KERNEL_TRACE", "0") == "1"
    try:
        res = run_bass_kernel_spmd(nc, in_maps, core_ids=list(range(NCORES)),
                                   trace=trace)
    except (ImportError, ModuleNotFoundError):
        # NTFF profiling hook unavailable in this environment
        res = run_bass_kernel_spmd(nc, in_maps, core_ids=list(range(NCORES)),
                                   trace=False)
    LAST_EXEC_NS = res.exec_time_ns
    LAST_RESULTS = res

    total = np.zeros((T, DIM), dtype=np.float32)
    for r in res.results:
        total += r["out_p"].astype(np.float32)
    return total.reshape(B, S, DIM)



# revision 46
# speedup vs baseline: 1.0175x; 1.0051x over previous
"""Multi-head attention (GQA prefill with KV cache) on 8 trn2 NeuronCores.

Sharding: tensor-parallel over heads. Core m owns KV head m (of 8) and the
4 query heads 4m..4m+3.  Each core computes its heads' attention output and
a partial x @ wo.T contribution; the host sums the 8 partials.

Layout notes:
  - All activations on device are "feature-major" ([feature, token]) so the
    token dim rides the matmul moving dim; x is transposed on the host once.
  - RoPE's (even, odd) pair interleave is removed by permuting wq/wk rows and
    cache_k's head_dim on the host (QK^T is invariant to a shared permutation
    of head_dim), so on device RoPE is plain 64-partition block arithmetic.
  - Scores are computed transposed ([key, query]) so the softmax denominator
    is an all-ones matmul and the attention output lands feature-major,
    which feeds the wo matmul directly.

Pipeline notes (v3, tuned against real NTFF profiles):
  - PSUM: qk 1 bank, sc 2x2-bank (paired score tiles), out 2, wo 1 = 16KB.
  - Attention kt-steps run in PAIRS: two scores matmuls share one
    [128,1024] PSUM tile so a single exp covers both, halving ACT
    instruction overhead; av matmuls trail one pair behind the scores so
    the PE never waits on the exp stream mid-unit.
  - Softmax denominators accumulate on DVE in fp16; an all-ones matmul
    broadcasts per-query sums; a fast ACT copy frees the PSUM slot and the
    slow DVE reciprocal runs quartered (progressive attnb availability)
    off the PE-critical path.  Each unit's epilogue is deferred two
    pair-steps into the next unit.
  - RoPE stages its PSUM input to fp16 SBUF via an ACT copy so the "qk"
    slot recycles without waiting on the DVE queue.
  - Emission is software-pipelined at sub-unit granularity: projection and
    wo matmul groups interleave between attention pair-steps, ALTERNATING
    between the two filler classes so single-banked PSUM slots have two
    steps to recycle.  wo staging copies ride ACT; quarter-row staging
    tiles double-buffer the output DMA.
  - Output partials are written bf16; the host sums the 8 core partials.
"""

import os
import sys

import numpy as np

if "/opt/trn_rl_repo" not in sys.path:
    sys.path.insert(0, "/opt/trn_rl_repo")

import ml_dtypes

import concourse.bass as bass
import concourse.mybir as mybir
import concourse.tile as tile
from concourse.bass_utils import run_bass_kernel_spmd
from concourse.masks import make_identity

BF16 = mybir.dt.bfloat16
FP16 = mybir.dt.float16
F32 = mybir.dt.float32
NP_BF16 = ml_dtypes.bfloat16

B, S, DIM = 4, 1024, 4096
N_HEADS, N_KV_HEADS = 32, 8
HD = 128
PAST = 1024
NCORES = 8
NQ = N_HEADS // NCORES  # 4 q heads per core
T = B * S  # 4096 tokens
DT = DIM // 128  # 32 contraction tiles
CH = 512  # projection token-chunk
NCH_B = S // CH  # chunks per batch (2)
SCW = 512  # attention s-chunk width
NKT = (PAST + S) // 128  # 16 key tiles per batch
ISQRT_HD = 1.0 / float(np.sqrt(HD))

LAST_EXEC_NS = None
LAST_RESULTS = None

_CACHED = {}


def _split_multi_waits(nc):
    """walrus' per-instruction sync encoding fits one wait; hoist extras
    onto standalone EventSemaphore instructions on the same engine queue."""
    for f in nc.m.functions:
        for blk in f.blocks:
            insts = blk.instructions
            if not any(i.sync_info and i.sync_info.on_wait
                       and len(i.sync_info.on_wait) > 1 for i in insts):
                continue
            new = []
            for inst in insts:
                si = inst.sync_info
                if si is not None and si.on_wait and len(si.on_wait) > 1:
                    waits = list(si.on_wait)
                    for wt in waits[:-1]:
                        evs = mybir.InstEventSemaphore(
                            name=f"I-wsplit-{nc.next_id()}", ins=[], outs=[])
                        evs.engine = inst.engine
                        evs.sync_info = mybir.SyncInfo(on_wait=[wt],
                                                       on_update=[])
                        new.append(evs)
                    inst.sync_info = mybir.SyncInfo(
                        on_wait=[waits[-1]],
                        on_update=list(si.on_update or []))
                new.append(inst)
            insts[:] = new


def _build_nc(split_waits=True, mode="full", reps=1):
    nc = bass.Bass("TRN2", target_bir_lowering=False, debug=False,
                   num_devices=NCORES)

    xt = nc.dram_tensor("xt", [DIM, T], BF16, kind="ExternalInput")
    wqt = nc.dram_tensor("wqt", [DIM, NQ * HD], BF16, kind="ExternalInput")
    wkt = nc.dram_tensor("wkt", [DIM, HD], BF16, kind="ExternalInput")
    wvt = nc.dram_tensor("wvt", [DIM, HD], BF16, kind="ExternalInput")
    wot = nc.dram_tensor("wot", [NQ * HD, DIM], BF16, kind="ExternalInput")
    ckt = nc.dram_tensor("ckt", [B, HD, PAST], BF16, kind="ExternalInput")
    cv = nc.dram_tensor("cv", [B, PAST, HD], BF16, kind="ExternalInput")
    cos = nc.dram_tensor("cos", [HD // 2, S], BF16, kind="ExternalInput")
    sin = nc.dram_tensor("sin", [HD // 2, S], BF16, kind="ExternalInput")
    out_p = nc.dram_tensor("out_p", [T, DIM], BF16, kind="ExternalOutput")

    with tile.TileContext(nc) as tc:
        if mode == "null":
            _emit_null(tc, nc)
        else:
            for _ in range(reps):
                _emit(tc, nc, xt, wqt, wkt, wvt, wot, ckt, cv, cos, sin,
                      out_p, mode=mode)
    if split_waits:
        _split_multi_waits(nc)
    return nc


def _emit_null(tc, nc):
    """Near-empty program: measures per-dispatch overhead in bench.py."""
    from contextlib import ExitStack
    with ExitStack() as ctx:
        cw = ctx.enter_context(tc.tile_pool(name="nullp", bufs=1))
        t = cw.tile([128, 128], BF16, name="null_t")
        nc.vector.memset(t, 0.0)


def _emit(tc, nc, xt, wqt, wkt, wvt, wot, ckt, cv, cos, sin, out_p,
          mode="full"):
    from contextlib import ExitStack
    do_attn = mode in ("full", "bc")
    do_wo = mode in ("full", "bd")

    with ExitStack() as ctx:
        cw = ctx.enter_context(tc.tile_pool(name="consts", bufs=1))
        pb = ctx.enter_context(tc.tile_pool(name="perbatch", bufs=2))
        wk = ctx.enter_context(tc.tile_pool(name="work", bufs=2))
        ps = ctx.enter_context(tc.tile_pool(name="ps", bufs=2, space="PSUM"))

        # ---- resident constants -------------------------------------------
        # DMA order is startup-critical: the first emitted work is the k/v
        # projection of batch 0 chunk 0, which needs only wkt/wvt (+ the xt
        # chunk, issued inside proj_batch).  wqt follows (needed ~15us later
        # by the first q unit); wot and the kv-cache much later.
        wkt_sb = cw.tile([128, DT * HD], BF16, name="wkt_sb")
        nc.sync.dma_start(
            out=wkt_sb.rearrange("p (n j) -> p n j", n=DT),
            in_=wkt[:, :].rearrange("(n p) j -> p n j", p=128))
        wvt_sb = cw.tile([128, DT * HD], BF16, name="wvt_sb")

        def load_wvt():
            nc.sync.dma_start(
                out=wvt_sb.rearrange("p (n j) -> p n j", n=DT),
                in_=wvt[:, :].rearrange("(n p) j -> p n j", p=128))
        # cos/sin duplicated across both 64-partition halves so RoPE's two
        # multiplies can run full-width: q*cos gives (r*cos | i*cos).
        # (loaded after the first xt chunk — see proj_batch)
        cos_sb = cw.tile([128, S], BF16, name="cos_sb")
        sin_sb = cw.tile([128, S], BF16, name="sin_sb")

        def load_cos_sin():
            nc.sync.dma_start(out=cos_sb[0:64, :], in_=cos[:, :])
            nc.sync.dma_start(out=cos_sb[64:128, :], in_=cos[:, :])
            nc.sync.dma_start(out=sin_sb[0:64, :], in_=sin[:, :])
            nc.sync.dma_start(out=sin_sb[64:128, :], in_=sin[:, :])

        ones128 = cw.tile([128, 128], FP16, name="ones128")
        nc.vector.memset(ones128, 1.0)
        ident = cw.tile([128, 128], BF16, name="ident")
        make_identity(nc, ident)
        wqt_sb = cw.tile([128, DT * NQ * HD], BF16, name="wqt_sb")
        wot_sb = cw.tile([128, NQ * DIM], BF16, name="wot_sb")

        def load_wqt():
            nc.sync.dma_start(
                out=wqt_sb.rearrange("p (n j) -> p n j", n=DT),
                in_=wqt[:, :].rearrange("(n p) j -> p n j", p=128))

        def load_wot():
            nc.sync.dma_start(
                out=wot_sb.rearrange("p (n d) -> p n d", n=NQ),
                in_=wot[:, :].rearrange("(n p) d -> p n d", p=128))

        # ---- per-batch / work tiles (slot handles, rotated via tags) ------
        def batch_tiles(b):
            qb_t = pb.tile([128, NQ * S], BF16, name="qb", tag="qb")
            kb_t = pb.tile([128, S], BF16, name="kb", tag="kb")
            vb_t = pb.tile([128, S], BF16, name="vb", tag="vb")
            attnb_t = pb.tile([128, NQ * S], BF16, name="attnb", tag="attnb",
                              bufs=1)
            ckt_b = pb.tile([128, PAST], BF16, name="ckt_b", tag="ckt_b")
            cv_b = pb.tile([128, PAST], BF16, name="cv_b", tag="cv_b")
            return dict(qb=qb_t, kb=kb_t, vb=vb_t, attnb=attnb_t,
                        ckt=ckt_b, cv=cv_b)

        def rope(dst_tile, dst_col, src_ps, cosc, sinc, n):
            """src layout (r|i) on partition halves.
            dst[0:64] = r*cos - i*sin ; dst[64:128] = r*sin + i*cos.
            A fast ACT copy stages src to fp16 SBUF first, freeing the "qk"
            PSUM slot immediately (the DVE rope ops otherwise queue behind
            attention acc-adds and stall the next proj group's matmuls).
            Two full-width muls: tc = (r*cos | i*cos), ts = (r*sin | i*sin),
            then dst_r = tc_hi - ts_lo, dst_i = ts_hi + tc_lo."""
            qraw = wk.tile([128, CH], FP16, name="qraw", tag="qraw", bufs=2)
            nc.scalar.copy(qraw[:, :n], src_ps)
            tc_ = wk.tile([128, CH], FP16, name="rope_tc", tag="rope_tc",
                          bufs=1)
            ts_ = wk.tile([128, CH], FP16, name="rope_ts", tag="rope_ts",
                          bufs=1)
            nc.vector.tensor_mul(tc_[:, :n], qraw[:, :n], cosc)
            nc.vector.tensor_mul(ts_[0:64, :n], qraw[64:128, :n],
                                 sinc[64:128, :])
            nc.vector.tensor_mul(ts_[64:128, :n], qraw[0:64, :n],
                                 sinc[0:64, :])
            nc.vector.tensor_sub(dst_tile[0:64, dst_col:dst_col + n],
                                 tc_[0:64, :n], ts_[0:64, :n])
            nc.vector.tensor_add(dst_tile[64:128, dst_col:dst_col + n],
                                 ts_[64:128, :n], tc_[64:128, :n])

        # ================= work-unit generators (each yields steps) ========
        def xt_load(b, c, after_piece=None):
            """DMA one 512-token chunk of x into SBUF (feature-major).
            Split so the projection's first accum matmuls can start as soon
            as the first piece lands (subtile deps).  after_piece maps piece
            index -> load callbacks interleaved onto the (serial) DMA ring
            by the time their consumer actually needs them."""
            xt_t = wk.tile([128, DT * CH], BF16, name="xt_t", tag="xt")
            p0 = c * CH
            # First chunk is latency-critical (PE idles until d0.. land):
            # progressive split so the first accum matmuls start ~7us in.
            splits = [2, 2, 4, 8, 16] if (b == 0 and c == 0) else [16, 16]
            d0 = 0
            for pi, step in enumerate(splits):
                nc.sync.dma_start(
                    out=xt_t[:, d0 * CH:(d0 + step) * CH].rearrange(
                        "p (n t) -> p n t", n=step),
                    in_=xt[128 * d0:128 * (d0 + step),
                           b * S + p0: b * S + p0 + CH].rearrange(
                        "(n p) t -> p n t", p=128))
                d0 += step
                for cb in (after_piece or {}).get(pi, ()):
                    cb()
            return xt_t

        def proj_unit(bt, b, c, kind, xt_t, alt=False):
            """One projection group: 32 accum matmuls + rope (or v-transpose).
            kind: 0..3 = q head j, 4 = k, 5 = v. Yields 5 steps.
            alt: prologue groups run back-to-back with no filler, so they
            alternate onto the (then idle) "out" bank instead of
            serializing on the single "qk" bank."""
            p0 = c * CH
            cosc = cos_sb[:, p0:p0 + CH]
            sinc = sin_sb[:, p0:p0 + CH]
            tag = "out" if (alt and kind % 2 == 1) else "qk"
            acc_ps = ps.tile([128, CH], F32, name="proj_ps", tag=tag,
                             bufs=2 if tag == "out" else 1)
            if kind < NQ:
                w_sb, wof = wqt_sb, kind * HD
                wstride = NQ * HD
            elif kind == NQ:
                w_sb, wof, wstride = wkt_sb, 0, HD
            else:
                w_sb, wof, wstride = wvt_sb, 0, HD
            for d0 in range(0, DT, 8):
                for d in range(d0, d0 + 8):
                    nc.tensor.matmul(
                        acc_ps,
                        lhsT=w_sb[:, d * wstride + wof:
                                  d * wstride + wof + HD],
                        rhs=xt_t[:, d * CH:(d + 1) * CH],
                        start=(d == 0), stop=(d == DT - 1))
                yield  # ~8 x 213ns PE quantum
            if kind < NQ:
                rope(bt["qb"], kind * S + p0, acc_ps, cosc, sinc, CH)
            elif kind == NQ:
                rope(bt["kb"], p0, acc_ps, cosc, sinc, CH)
            else:
                vcp = wk.tile([128, CH], BF16, name="vcp", tag="vcp", bufs=1)
                nc.scalar.copy(vcp, acc_ps)
                for tsub in range(CH // 128):
                    vtr_ps = ps.tile([128, 128], BF16, name="vtr_ps",
                                     tag="sc")
                    nc.tensor.transpose(vtr_ps,
                                        vcp[:, tsub * 128:(tsub + 1) * 128],
                                        ident)
                    col = (c * (CH // 128) + tsub) * 128
                    nc.vector.tensor_copy(bt["vb"][:, col:col + 128], vtr_ps)
            yield

        def attn_unit(bt, b, sc, h, epi_out):
            """Attention for (query-chunk sc, head h): 16 kt steps.
            Scores land [key, query] in PSUM; exp on ACT; denominator
            accumulates on DVE in fp16; all-ones matmul broadcasts sums.

            Pipelining (v3, NTFF-driven):
              - av(kt) is emitted one step behind scores(kt+1), so the PE
                never waits on the ACT exp stream mid-unit.
              - The epilogue (sums broadcast + reciprocal + normalize) is
                handed back via epi_out and emitted ~2 kt-steps into the
                NEXT unit, so its ACT copy does not delay exp(kt0') and the
                sums matmuls get runway before reading the DVE accs."""
            s0 = sc * SCW
            out_ps = ps.tile([128, SCW], F32, name="out_ps", tag="out")
            acc0 = wk.tile([128, SCW], FP16, name="acc0", tag="acc0", bufs=1)
            acc1 = wk.tile([128, SCW], FP16, name="acc1", tag="acc1", bufs=1)
            qsl = bt["qb"][:, h * S + s0:h * S + s0 + SCW]

            def kv(kt):
                if kt < PAST // 128:
                    return (bt["ckt"][:, kt * 128:(kt + 1) * 128],
                            bt["cv"][:, kt * 128:(kt + 1) * 128])
                kn = kt - PAST // 128
                return (bt["kb"][:, kn * 128:(kn + 1) * 128],
                        bt["vb"][:, kn * 128:(kn + 1) * 128])

            # kt-steps processed in PAIRS sharing one 2-bank PSUM tile so a
            # single [128,1024] exp covers both — halves ACT instruction
            # count/overhead (the exp stream otherwise paces attention).
            prev = None
            for p in range(NKT // 2):
                k0, v0 = kv(2 * p)
                k1, v1 = kv(2 * p + 1)
                sc_ps = ps.tile([128, 2 * SCW], F32, name="sc_ps", tag="sc")
                nc.tensor.matmul(sc_ps[:, 0:SCW], lhsT=k0, rhs=qsl)
                nc.tensor.matmul(sc_ps[:, SCW:2 * SCW], lhsT=k1, rhs=qsl)
                if prev is not None:
                    pexp, pv0, pv1 = prev
                    nc.tensor.matmul(out_ps, lhsT=pv0, rhs=pexp[:, 0:SCW],
                                     start=(p == 1), stop=False)
                    nc.tensor.matmul(out_ps, lhsT=pv1,
                                     rhs=pexp[:, SCW:2 * SCW],
                                     start=False, stop=False)
                exp_t = wk.tile([128, 2 * SCW], BF16, name="exp_t",
                                tag="exp", bufs=2)
                nc.scalar.activation(exp_t, sc_ps,
                                     mybir.ActivationFunctionType.Exp,
                                     scale=ISQRT_HD)
                if p == 0:
                    nc.vector.tensor_copy(acc0, exp_t[:, 0:SCW])
                    nc.vector.tensor_copy(acc1, exp_t[:, SCW:2 * SCW])
                else:
                    nc.vector.tensor_add(acc0, acc0, exp_t[:, 0:SCW])
                    nc.vector.tensor_add(acc1, acc1, exp_t[:, SCW:2 * SCW])
                prev = (exp_t, v0, v1)
                yield
            pexp, pv0, pv1 = prev
            nc.tensor.matmul(out_ps, lhsT=pv0, rhs=pexp[:, 0:SCW],
                             start=False, stop=False)
            nc.tensor.matmul(out_ps, lhsT=pv1, rhs=pexp[:, SCW:2 * SCW],
                             start=False, stop=True)

            def epilogue():
                sums_ps = ps.tile([128, SCW], F32, name="sums_ps", tag="sc")
                nc.tensor.matmul(sums_ps, lhsT=ones128, rhs=acc0,
                                 start=True, stop=False)
                nc.tensor.matmul(sums_ps, lhsT=ones128, rhs=acc1,
                                 start=False, stop=True)
                # Fast ACT copy frees the "sc" PSUM slot quickly; the
                # reciprocal (3.4us on real DVE) runs in-place afterwards,
                # off the PE-critical path — quartered so the first attnb
                # 128-token tile lands ~3us sooner (wo rows consume attnb
                # per 128-token tile and otherwise stall on batch 3's tail).
                inv_bc = wk.tile([128, SCW], BF16, name="inv_bc", tag="inv",
                                 bufs=1)
                nc.scalar.copy(inv_bc, sums_ps)
                for qtr in range(SCW // 128):
                    sl = slice(qtr * 128, (qtr + 1) * 128)
                    with nc.allow_low_precision(reason="softmax denom recip"):
                        nc.vector.reciprocal(inv_bc[:, sl], inv_bc[:, sl])
                    nc.vector.tensor_mul(
                        bt["attnb"][:, h * S + s0 + qtr * 128:
                                    h * S + s0 + (qtr + 1) * 128],
                        out_ps[:, sl], inv_bc[:, sl])
            epi_out.append(epilogue)
            yield

        NDC = DIM // SCW  # 8 wo column-groups per token tile
        HROW = NDC // 4   # groups per staging tile (quarter row)

        def wo_unit(bt, b, tt, dc, st_half, alt):
            """Partial x@wo.T for token-tile tt, 512 output dims dc.
            alt: when the proj fillers can no longer touch "qk" (all of the
            last batch, each batch's final chunk), odd groups borrow that
            idle bank so back-to-back wo groups don't serialize on the
            single "wo" bank waiting for the staging copy."""
            tag = "qk" if (alt and dc % 2 == 1) else "wo"
            wo_ps = ps.tile([128, SCW], F32, name="wo_ps", tag=tag,
                            bufs=1)
            for j in range(NQ):
                nc.tensor.matmul(
                    wo_ps,
                    lhsT=bt["attnb"][:, j * S + tt * 128:
                                     j * S + (tt + 1) * 128],
                    rhs=wot_sb[:, j * DIM + dc * SCW:
                               j * DIM + (dc + 1) * SCW],
                    start=(j == 0), stop=(j == NQ - 1))
            # GPSIMD cannot read PSUM.  All staging copies go to ACT: with
            # paired exps ACT is ~45% loaded, while the DVE queue carries
            # the epilogue reciprocal bursts that would delay the copy and
            # stall the single-banked wo_ps slot (measured via NTFF).
            dst = st_half[:, (dc % HROW) * SCW:(dc % HROW + 1) * SCW]
            nc.scalar.copy(dst, wo_ps)
            if dc % HROW == HROW - 1:
                row = (b * (S // 128) + tt) * 128
                col = (dc // HROW) * HROW * SCW
                nc.sync.dma_start(
                    out=out_p[row:row + 128, col:col + HROW * SCW],
                    in_=st_half)
            yield

        def wo_row(bt, b, tt, alt):
            for part in range(NDC // HROW):
                st_half = wk.tile([128, HROW * SCW], BF16, name="st_half",
                                  tag="st", bufs=2)
                for dc in range(part * HROW, (part + 1) * HROW):
                    yield from wo_unit(bt, b, tt, dc, st_half, alt)

        # ================= scheduler ======================================
        # Per batch: proj(b) is fully emitted before attn(b).  attn(b)'s 8
        # units are ACT-paced; between kt-steps we interleave filler steps
        # from wo(b) rows (ready once all heads of their s-chunk are done)
        # and proj(b+1) (+ its xt prefetch).  All wo(b) rows are emitted
        # before attn(b+1) starts so attnb can be single-buffered.
        bts = [None] * B
        xts = [None] * B

        def cache_load(b):
            bt = bts[b]
            nc.sync.dma_start(out=bt["ckt"], in_=ckt[b, :, :])
            nc.sync.dma_start(
                out=bt["cv"].rearrange("p (n d) -> p n d", n=PAST // 128),
                in_=cv[b, :, :].rearrange("(n p) t -> p n t", p=128))

        def proj_batch(b):
            """Generator over proj(b) steps (incl. xt loads + cache DMA).
            The kv-cache DMA is issued after the first xt chunk so it does
            not delay the projection-critical loads on the in-order queue.
            Heads q2/q3 are deferred (see projq23_batch): they are only
            needed by attention units h2/h3, so they serve as attn-phase
            filler — which matters for the last batch, whose attention has
            no next-batch projections to interleave."""
            bts[b] = batch_tiles(b)
            xts[b] = [None] * NCH_B
            for c in range(NCH_B):
                xts[b][c] = xt_load(b, c)
                if b == 0:
                    if c == 0:
                        load_wvt()
                        load_cos_sin()
                        load_wqt()
                    else:
                        load_wot()
                        cache_load(b)
                elif c == 0:
                    cache_load(b)
                for kind in (NQ, NQ + 1, 0, 1):  # k, v first
                    yield from proj_unit(bts[b], b, c, kind, xts[b][c],
                                         alt=(b == 0))

        def projq23_batch(b):
            for c in range(NCH_B):
                for kind in (2, 3):
                    yield from proj_unit(bts[b], b, c, kind, xts[b][c])

        def attn_batch(b):
            epis = []        # deferred epilogue of the previous unit
            pending_sc = None  # sc-chunk whose last epilogue is deferred
            for sc in range(S // SCW):
                for h in range(NQ if do_attn else 0):
                    step_n = 0
                    for _ in attn_unit(bts[b], b, sc, h, epis):
                        yield
                        step_n += 1
                        if step_n == 2 and epis:
                            epis.pop(0)()
                            if pending_sc is not None:
                                yield ("sc_done", pending_sc)
                                pending_sc = None
                pending_sc = sc
            while epis:
                epis.pop(0)()
            if pending_sc is not None:
                yield ("sc_done", pending_sc)

        def wo_rows(b, tts, alt=False):
            for tt in tts:
                yield from wo_row(bts[b], b, tt, alt)

        # prologue: batch 0 projections (k/v/q0/q1), no filler available
        for _ in proj_batch(0):
            pass

        for b in range(B):
            # Filler queues: proj-ish (deferred q2/q3 of batch b, then
            # proj(b+1)) and wo rows of batch b (gated on sc completion).
            # Strict FIFO within proj-ish (q23 before proj(b+1)): proj(b+1)'s
            # xt DMA reuses q23(b)'s xt slot, so emitting it before all
            # q23(b) matmuls are on the PE queue would deadlock the in-order
            # queues.  Between classes we ALTERNATE: consecutive proj groups
            # otherwise stall ~3us each on the "qk" PSUM slot, whose freeing
            # ACT copy sits behind the exp stream (measured via NTFF).
            pf = [projq23_batch(b)]
            if b + 1 < B:
                pf.append(proj_batch(b + 1))
            wf = []
            rr = [0]

            def drain_filler(n, pf=pf, wf=wf, rr=rr):
                done = 0
                while done < n and (pf or wf):
                    use_wf = bool(wf) and (not pf or rr[0] % 2 == 1)
                    rr[0] += 1
                    gen = wf[0] if use_wf else pf[0]
                    try:
                        next(gen)
                        done += 1
                    except StopIteration:
                        (wf if use_wf else pf).remove(gen)
                return done

            if do_attn:
                for step in attn_batch(b):
                    if isinstance(step, tuple) and step[0] == "sc_done":
                        sc = step[1]
                        if do_wo:
                            tts = range(sc * (SCW // 128),
                                        (sc + 1) * (SCW // 128))
                            alt = (b == B - 1) or (sc == S // SCW - 1)
                            wf.append(wo_rows(b, tts, alt))
                        continue
                    # one attn pair-step emitted (4 matmuls); ~2 filler
                    # steps.  The last batch has only ~84 filler steps for
                    # 72 yields — drain 1 so they stretch across the batch.
                    drain_filler(2 if b + 1 < B else 1)
            elif do_wo:
                wf.append(wo_rows(b, range(S // 128), alt=True))
            # drain all remaining wo(b) (+ any proj(b+1) leftovers)
            while drain_filler(64):
                pass


def _rope_perm():
    # even features first, then odd — per 128-wide head
    return np.concatenate([np.arange(0, HD, 2), np.arange(1, HD, 2)])


def _prep_inputs(x, freqs_cos, freqs_sin, cache_k, cache_v, wq, wk, wv, wo):
    perm = _rope_perm()
    xt = np.ascontiguousarray(
        x.reshape(T, DIM).T).astype(NP_BF16)
    cos_t = np.ascontiguousarray(freqs_cos.T).astype(NP_BF16)
    sin_t = np.ascontiguousarray(freqs_sin.T).astype(NP_BF16)

    in_maps = []
    for m in range(NCORES):
        wq_m = wq[m * NQ * HD:(m + 1) * NQ * HD]  # (512, 4096)
        wq_m = wq_m.reshape(NQ, HD, DIM)[:, perm, :].reshape(NQ * HD, DIM)
        wqt_m = np.ascontiguousarray(wq_m.T).astype(NP_BF16)
        wk_m = wk[m * HD:(m + 1) * HD][perm]
        wkt_m = np.ascontiguousarray(wk_m.T).astype(NP_BF16)
        wv_m = wv[m * HD:(m + 1) * HD]
        wvt_m = np.ascontiguousarray(wv_m.T).astype(NP_BF16)
        wot_m = np.ascontiguousarray(
            wo[:, m * NQ * HD:(m + 1) * NQ * HD].T).astype(NP_BF16)
        ckt_m = np.ascontiguousarray(
            cache_k[:, m][:, :, perm].transpose(0, 2, 1)).astype(NP_BF16)
        cv_m = np.ascontiguousarray(cache_v[:, m]).astype(NP_BF16)
        in_maps.append({
            "xt": xt, "wqt": wqt_m, "wkt": wkt_m, "wvt": wvt_m,
            "wot": wot_m, "ckt": ckt_m, "cv": cv_m,
            "cos": cos_t, "sin": sin_t,
        })
    return in_maps


def kernel(x, freqs_cos, freqs_sin, cache_k, cache_v, wq, wk, wv, wo):
    global LAST_EXEC_NS, LAST_RESULTS
    if "nc" not in _CACHED:
        _CACHED["nc"] = _build_nc()
    nc = _CACHED["nc"]

    in_maps = _prep_inputs(np.asarray(x), np.asarray(freqs_cos),
                           np.asarray(freqs_sin), np.asarray(cache_k),
                           np.asarray(cache_v), np.asarray(wq),
                           np.asarray(wk), np.asarray(wv), np.asarray(wo))

    trace = os.environ.get("KERNEL_TRACE", "0") == "1"
    try:
        res = run_bass_kernel_spmd(nc, in_maps, core_ids=list(range(NCORES)),
                                   trace=trace)
    except (ImportError, ModuleNotFoundError):
        # NTFF profiling hook unavailable in this environment
        res = run_bass_kernel_spmd(nc, in_maps, core_ids=list(range(NCORES)),
                                   trace=False)
    LAST_EXEC_NS = res.exec_time_ns
    LAST_RESULTS = res

    total = np.zeros((T, DIM), dtype=np.float32)
    for r in res.results:
        total += r["out_p"].astype(np.float32)
    return total.reshape(B, S, DIM)

